# revision 2
# baseline (speedup 1.0000x reference)
"""Block-causal attention (B=2, S=2048, D=1024, H=16, HD=64, BLOCK=16) on 8 TRN2 cores.

Sharding: core c -> batch c//4, head-group c%4 (4 heads). Each core computes the
full attention for its 4 heads plus a partial out-projection y^T (1024, 2048) in
f16; the host sums the 4 partials per batch and transposes.

v2 restructure vs baseline:
  - xt DMA in column blocks (arrival order == consumption order) so the Q0
    projection starts ~2us in.
  - proj order Q0,K0 -> lnexp group0 -> V -> Q1,K1 -> lnexp group1. RMS-norm
    Ln/Exp merged per group: pairs live at 32-partition offsets in one psum
    tile, one Ln + one Exp on [34, 2048] instead of 4 instructions apiece.
  - squares on ACT (Square), psum->sbuf raw copies split DVE/ACT to balance
    engines under the PE roofline.
  - attention qh-outer with the two heads of an mt pair interleaved per
    k-tile: their K=64 score matmuls sit at tile_position (0,0)/(64,0) and
    run concurrently on HW; second mask-factor copy lives at partitions 64-71.
  - av evacuated to SBUF by DVE right after the last PV so the psum bank
    frees early; normalize runs from the SBUF copy.
  - out-proj jh=0 interleaved into the qh=1 attention stream (shared score
    psum pool); output yt in f16 (host sums partials in f32).
"""

import numpy as np
import ml_dtypes

import concourse.bass as bass
import concourse.tile as tile
from concourse import bacc
from concourse import mybir
from concourse.bass_utils import run_bass_kernel_spmd

BF16 = ml_dtypes.bfloat16
F32 = mybir.dt.float32
F16 = mybir.dt.float16
BF = mybir.dt.bfloat16

B, S, D, H, HD = 2, 2048, 1024, 16, 64
HLOC = 4          # heads per core
NCORES = 8
EPS = 1e-6
SCALE = HD ** -0.5
MASK_C = 8192.0   # masked-pair score offset; exp underflows to 0.0
NST = 4           # 512-wide seq tiles
NKT = 16          # 128-wide key tiles
NDK = 8           # 128-wide model-dim tiles


def _declare_io(nc):
    def din(name, shape, d=BF):
        return nc.dram_tensor(name, shape, d, kind="ExternalInput").ap()

    io = dict(
        xt_d=din("xt", [D, S]),
        wq_d=din("wq", [128, NDK * 256]),
        wk_d=din("wk", [128, NDK * 256]),
        wv_d=din("wv", [128, NDK * 256]),
        wo_d=din("wo", [128, 2 * D]),
        csq_d=din("csq", [64, S]),
        snq_d=din("snq", [64, S]),
        csk_d=din("csk", [64, S]),
        snk_d=din("snk", [64, S]),
        mu_d=din("mu", [8, 128]),
        mv_d=din("mv", [8, 128]),
        ones2_d=din("ones2", [128, 2]),
        b0_d=din("b0", [128, 1], F32),
        yt_d=nc.dram_tensor(
            "yt", [32, 128, 512], F16, kind="ExternalOutput"
        ).ap(),
    )
    return io


def _emit(tc, io, u=""):
    """Emit the per-core program. Pure SPMD: identical on all 8 cores."""
    from contextlib import ExitStack

    nc = tc.nc
    A = mybir.ActivationFunctionType
    xt_d = io["xt_d"]
    wo_d = io["wo_d"]
    mu_d = io["mu_d"]
    mv_d = io["mv_d"]
    ones2_d = io["ones2_d"]
    b0_d = io["b0_d"]
    yt_d = io["yt_d"]

    ctx = ExitStack()
    proj_ctx = ExitStack()
    with ctx:
        consts = ctx.enter_context(tc.tile_pool(name="consts" + u, bufs=1))
        persist = ctx.enter_context(tc.tile_pool(name="persist" + u, bufs=1))
        dscratch = ctx.enter_context(
            tc.tile_pool(name="dscratch" + u, bufs=1, space="DRAM")
        )
        xtp = proj_ctx.enter_context(tc.tile_pool(name="xtp" + u, bufs=1))
        work2 = proj_ctx.enter_context(tc.tile_pool(name="work2" + u, bufs=2))
        sqp = proj_ctx.enter_context(tc.tile_pool(name="sqp" + u, bufs=3))
        pp = proj_ctx.enter_context(
            tc.tile_pool(name="pp" + u, bufs=2, space="PSUM")
        )
        msp = proj_ctx.enter_context(
            tc.tile_pool(name="msp" + u, bufs=2, space="PSUM")
        )

        # ---- input DMA: xt in column blocks (2 chunks per 512-col block)
        # on the sync/scalar HWDGE queues; weights+tables on the gpsimd
        # SWDGE queue in consumption order ----
        xt_sb = xtp.tile([128, NDK, S], BF)
        xt_rd = xt_d.rearrange("(a p) c -> p a c", p=128)
        for st in range(NST):
            sl = slice(512 * st, 512 * (st + 1))
            nc.sync.dma_start(out=xt_sb[:, 0:4, sl], in_=xt_rd[:, 0:4, sl])
            nc.scalar.dma_start(out=xt_sb[:, 4:8, sl], in_=xt_rd[:, 4:8, sl])

        wq_sb = consts.tile([128, NDK, 256], BF)
        wk_sb = consts.tile([128, NDK, 256], BF)
        wv_sb = consts.tile([128, NDK, 256], BF)
        wo_sb = consts.tile([128, 2, D], BF)
        for nm, t in (("wv", wv_sb), ("wq", wq_sb), ("wk", wk_sb)):
            nc.gpsimd.dma_start(
                out=t, in_=io[nm + "_d"].rearrange("p (t m) -> p t m", t=NDK)
            )
        # rope tables: DRAM holds 64 rows; duplicate into both SBUF halves
        csq_sb = consts.tile([128, S], BF)
        snq_sb = consts.tile([128, S], BF)
        csk_sb = consts.tile([128, S], BF)
        snk_sb = consts.tile([128, S], BF)
        for nm, t in (
            ("csq", csq_sb), ("snq", snq_sb), ("csk", csk_sb), ("snk", snk_sb)
        ):
            nc.gpsimd.dma_start(out=t[0:64], in_=io[nm + "_d"])
            nc.gpsimd.dma_start(out=t[64:128], in_=io[nm + "_d"])
        nc.gpsimd.dma_start(out=wo_sb, in_=wo_d.rearrange("p (t m) -> p t m", t=2))
        # rank-8 mask factors at partitions 0-7 (heads at po=0) and a second
        # copy at partitions 64-71 (heads at po=64) for PE row-group overlap
        mu_sb = consts.tile([8, 128], BF)
        mv_sb = consts.tile([8, 128], BF)
        mm64 = consts.tile([72, 2, 128], BF)
        nc.sync.dma_start(out=mu_sb, in_=mu_d)
        nc.sync.dma_start(out=mv_sb, in_=mv_d)
        nc.sync.dma_start(out=mm64[64:72, 0, :], in_=mu_d)
        nc.sync.dma_start(out=mm64[64:72, 1, :], in_=mv_d)
        ones2_sb = consts.tile([128, 2], BF)
        nc.sync.dma_start(out=ones2_sb, in_=ones2_d)
        b0_sb = consts.tile([128, 1], F32)
        nc.sync.dma_start(out=b0_sb, in_=b0_d)
        eps_sb = consts.tile([128, 1], F32)
        nc.vector.memset(eps_sb, EPS)

        # ---- persistent activations ----
        qT = persist.tile([128, 2, S], BF)      # (2 heads)*64 rows per m-tile
        kT = persist.tile([128, 2, S], BF)
        vv = persist.tile([128, NKT, HLOC, HD + 1], BF)   # [V | ones]
        at = persist.tile([128, 2, S], BF)      # normalized attn^T
        # pair p's two rrms rows live at partition 32*p
        ln8 = persist.tile([98, NST, 512], F32)
        rr8 = persist.tile([98, NST, 512], BF)
        rkb = persist.tile([128, 64], BF)    # k-side rrms, (k mod 128, h*16+i)
        rkz = persist.tile([128, 4, 16], F32)  # SCALE * rrms_k per (head, ktile)
        rr_dram = dscratch.tile([8, 16, 128], BF)

        nc.vector.memset(vv[:, :, :, HD : HD + 1], 1.0)

        # ---- phase 1: Q/K projections + RMS-norm stats + RoPE ----
        def proj_pair(qk_i, mt):
            """Project pair (qk_i: 0=Q, 1=K) for m-tile mt; fill qraw, compute
            rrms = exp(-0.5*ln(ms/HD+eps)) into rr8. Returns the qraw tile."""
            pair = 2 * mt + qk_i
            pb = 32 * pair
            wsb = wq_sb if qk_i == 0 else wk_sb
            qraw = work2.tile([128, S], BF, tag="qraw")
            for hf in range(2):
                ms_t = msp.tile([2, 2, 512], F32, tag="ms")
                for s2 in range(2):
                    st = 2 * hf + s2
                    ps = pp.tile([128, 512], F32, tag="pp")
                    for kt in range(NDK):
                        nc.tensor.matmul(
                            ps,
                            lhsT=wsb[:, kt, 128 * mt : 128 * (mt + 1)],
                            rhs=xt_sb[:, kt, 512 * st : 512 * (st + 1)],
                            start=(kt == 0),
                            stop=(kt == NDK - 1),
                        )
                    sl = slice(512 * st, 512 * (st + 1))
                    if st % 2 == 0:
                        nc.vector.tensor_copy(qraw[:, sl], ps)
                    else:
                        nc.scalar.copy(qraw[:, sl], ps)
                    sq = sqp.tile([128, 512], BF, tag="sq")
                    nc.scalar.activation(sq, qraw[:, sl], A.Square)
                    nc.tensor.matmul(
                        ms_t[:, s2, :],
                        lhsT=ones2_sb,
                        rhs=sq,
                        start=True,
                        stop=True,
                    )
                nc.scalar.activation(
                    ln8[pb : pb + 2, 2 * hf : 2 * hf + 2, :],
                    ms_t,
                    A.Ln,
                    bias=eps_sb[0:2],
                    scale=1.0 / HD,
                )
                nc.scalar.activation(
                    rr8[pb : pb + 2, 2 * hf : 2 * hf + 2, :],
                    ln8[pb : pb + 2, 2 * hf : 2 * hf + 2, :],
                    A.Exp,
                    scale=-0.5,
                )
            return qraw

        def rope(qk_i, mt, qraw):
            """RoPE on raw projection output. Q side: multiply by the
            broadcast rrms afterwards; K side: plain add (rrms folded into
            the exp scale)."""
            pair = 2 * mt + qk_i
            pb = 32 * pair
            cstab = csq_sb if qk_i == 0 else csk_sb
            sntab = snq_sb if qk_i == 0 else snk_sb
            dest = qT if qk_i == 0 else kT
            rot = work2.tile([128, S], BF, tag="rot")
            for lo, hi in ((0, 32), (32, 64), (64, 96), (96, 128)):
                src_lo = lo + 32 if (lo // 32) % 2 == 0 else lo - 32
                eng = nc.sync if lo < 64 else nc.scalar
                eng.dma_start(out=rot[lo:hi], in_=qraw[src_lo : src_lo + 32])
            t1 = work2.tile([128, S], BF, tag="t1")
            t2 = work2.tile([128, S], BF, tag="t2")
            nc.vector.tensor_mul(t1, qraw, cstab)
            nc.vector.tensor_mul(t2, rot, sntab)
            nc.sync.dma_start(
                out=rr_dram[2 * pair : 2 * pair + 2].rearrange("r a b -> r (a b)"),
                in_=rr8[pb : pb + 2].rearrange("p a b -> p (a b)"),
            )
            if qk_i == 1:
                nc.vector.tensor_add(dest[:, mt, :], t1, t2)
                # k-side rrms rows -> partition-major via DMA transpose,
                # folding in the 1/sqrt(HD) softmax scale
                nc.sync.dma_start_transpose(
                    rkb[:, 32 * mt : 32 * (mt + 1)],
                    rr_dram[2 * pair : 2 * pair + 2].rearrange("r a b -> (r a) b"),
                )
                nc.vector.tensor_scalar_mul(
                    rkz[:, 2 * mt : 2 * mt + 2, :].rearrange("p h i -> p (h i)"),
                    rkb[:, 32 * mt : 32 * (mt + 1)],
                    SCALE,
                )
            else:
                tsum = work2.tile([128, S], BF, tag="tsum")
                nc.vector.tensor_add(tsum, t1, t2)
                rrb = work2.tile([128, NST, 512], BF, tag="rrb")
                nc.gpsimd.dma_start(
                    out=rrb[0:64],
                    in_=rr_dram[2 * pair : 2 * pair + 1]
                    .rearrange("r a b -> r (a b)")
                    .rearrange("r (a b) -> r a b", a=NST)
                    .partition_broadcast(64),
                )
                nc.gpsimd.dma_start(
                    out=rrb[64:128],
                    in_=rr_dram[2 * pair + 1 : 2 * pair + 2]
                    .rearrange("r a b -> r (a b)")
                    .rearrange("r (a b) -> r a b", a=NST)
                    .partition_broadcast(64),
                )
                for st in range(NST):
                    sl = slice(512 * st, 512 * (st + 1))
                    nc.vector.tensor_mul(
                        dest[:, mt, sl], tsum[:, sl], rrb[:, st, :]
                    )

        # ---- phase 1a: V projection first — it only needs xt, so it rides
        # the tail of the input DMA; two st tiles per psum bank ----
        for sp2 in range(NKT // 2):
            ps = pp.tile([128, 512], F32, tag="pp")
            for half in range(2):
                stv = 2 * sp2 + half
                for kt in range(NDK):
                    nc.tensor.matmul(
                        ps[:, 256 * half : 256 * (half + 1)],
                        lhsT=xt_sb[:, kt, 128 * stv : 128 * (stv + 1)],
                        rhs=wv_sb[:, kt, :],
                        start=(kt == 0),
                        stop=(kt == NDK - 1),
                    )
            nc.vector.tensor_copy(
                vv[:, 2 * sp2 : 2 * sp2 + 2, :, 0:HD],
                ps.rearrange("p (a h d) -> p a h d", a=2, h=HLOC),
            )

        for mt in range(2):
            qraw_q = proj_pair(0, mt)
            rope(0, mt, qraw_q)
            qraw_k = proj_pair(1, mt)
            rope(1, mt, qraw_k)

        # proj scratch (incl. x^T) is dead now; free SBUF/PSUM for attention
        proj_ctx.close()
        attnw = ctx.enter_context(tc.tile_pool(name="attnw" + u, bufs=3))
        ptp = ctx.enter_context(tc.tile_pool(name="ptp" + u, bufs=6))
        ystp = ctx.enter_context(tc.tile_pool(name="ystp" + u, bufs=3))
        spp = ctx.enter_context(tc.tile_pool(name="spp" + u, bufs=2, space="PSUM"))
        avp = ctx.enter_context(tc.tile_pool(name="avp" + u, bufs=2, space="PSUM"))

        def attn_se(h, qh, i):
            """Scores + exp for head h, query-half qh, k-tile i -> pt."""
            mt, half = divmod(h, 2)
            po = 64 * half
            glo = 1024 * qh
            q0 = 128 * i
            lo_g = max(glo, q0)
            pt = ptp.tile([128, 1024], BF, tag="pt")
            sp = spp.tile([128, 1024], F32, tag="spp")
            has_diag = glo <= q0 < glo + 1024
            for jj in range(2):
                j = 2 * qh + jj
                lo = max(512 * j, q0)
                hi = 512 * (j + 1)
                if lo >= hi:
                    continue
                diag_bank = has_diag and (q0 - glo) // 512 == jj
                nc.tensor.matmul(
                    sp[:, lo - glo : hi - glo],
                    lhsT=kT[po : po + 64, mt, 128 * i : 128 * (i + 1)],
                    rhs=qT[po : po + 64, mt, lo:hi],
                    start=True,
                    stop=not diag_bank,
                )
                if diag_bank:
                    # block-causal mask: scores -= MASK_C * disallowed
                    mum = mu_sb if half == 0 else mm64[64:72, 0, :]
                    mvm = mv_sb if half == 0 else mm64[64:72, 1, :]
                    nc.tensor.matmul(
                        sp[:, q0 - glo : q0 - glo + 128],
                        lhsT=mum,
                        rhs=mvm,
                        start=False,
                        stop=True,
                    )
            # P^T = exp(rrms_k[k]/sqrt(HD) * scores - B0)
            nc.scalar.activation(
                pt[:, lo_g - glo : 1024],
                sp[:, lo_g - glo : 1024],
                A.Exp,
                bias=b0_sb,
                scale=rkz[:, h, i : i + 1],
            )
            return pt

        def attn_pv(h, qh, i, av, pt):
            """attn^T accumulation (+ denominator in row 64)."""
            glo = 1024 * qh
            kmax = 8 * (qh + 1)
            q0 = 128 * i
            for jj in range(2):
                j = 2 * qh + jj
                jlo = max(512 * j, q0)
                jhi = 512 * (j + 1)
                if jlo >= jhi:
                    continue
                nc.tensor.matmul(
                    av[:, jj, jlo - 512 * j : 512],
                    lhsT=vv[:, i, h, :],
                    rhs=pt[:, jlo - glo : jhi - glo],
                    start=(i == 0),
                    stop=(i == min(kmax, 4 * j + 4) - 1),
                )

        def normalize(h, qh, av, jj):
            """Evacuate one jj half of av to SBUF (its accumulation closed at
            i=4j+3), then divide the 64 head rows by the denominator row.
            Splitting by jj lets half the work run inside the i loop and
            frees the psum bank sooner at group end."""
            mt, half = divmod(h, 2)
            po = 64 * half
            lo = 1024 * qh + 512 * jj
            avs = attnw.tile([65, 512], F32, tag="avs")
            nc.vector.tensor_copy(avs, av[:, jj, :])
            rden = attnw.tile([1, 512], F32, tag="rden")
            nc.vector.reciprocal(rden, avs[64:65])
            rdb = attnw.tile([64, 512], F32, tag="rdb")
            nc.gpsimd.partition_broadcast(rdb, rden, channels=64)
            nc.vector.tensor_mul(
                at[po : po + 64, mt, lo : lo + 512],
                avs[0:64],
                rdb,
            )

        def outproj_m(jh, m):
            """One 128-row tile of the partial out-projection y^T."""
            ps = spp.tile([128, 1024], F32, tag="spp")
            for jj in range(2):
                j = 2 * jh + jj
                for kt in range(2):
                    nc.tensor.matmul(
                        ps[:, 512 * jj : 512 * (jj + 1)],
                        lhsT=wo_sb[:, kt, 128 * m : 128 * (m + 1)],
                        rhs=at[:, kt, 512 * j : 512 * (j + 1)],
                        start=(kt == 0),
                        stop=(kt == 1),
                    )
            yst = ystp.tile([128, 2, 512], F16, tag="yst")
            # jh=0 runs inside the attention window where ACT is the
            # bottleneck -> DVE; jh=1 is the tail where ACT idles -> mostly ACT
            # (DVE carries the trailing normalizes there).
            if jh == 1 and m % 2 == 1:
                nc.scalar.copy(yst, ps.rearrange("p (a b) -> p a b", a=2))
            else:
                nc.vector.tensor_copy(yst, ps.rearrange("p (a b) -> p a b", a=2))
            eng = nc.sync if m % 2 == 0 else nc.scalar
            eng.dma_start(
                out=yt_d[4 * m + 2 * jh : 4 * m + 2 * jh + 2].rearrange(
                    "a p b -> p a b"
                ),
                in_=yst,
            )

        # ---- phase 3: attention, qh-outer, head pairs interleaved, with a
        # one-step software pipeline: scores+exp of step N issue before the
        # PV of step N-1, so group boundaries never starve the ACT engine ----
        steps = []
        for qh in range(2):
            for mt in range(2):
                gav = {}
                for i in range(8 * (qh + 1)):
                    for h in (2 * mt, 2 * mt + 1):
                        steps.append((h, qh, mt, i, gav))

        def post_pv(h, qh, mt, i, gav):
            """Hooks that must follow the PV of (h, qh, i)."""
            hA, hB = 2 * mt, 2 * mt + 1
            if h != hB:
                return
            kmax = 8 * (qh + 1)
            jj0_done = 3 if qh == 0 else 11
            if i == jj0_done:
                # first query-half of av closed -> normalize it now
                normalize(hA, qh, gav[hA], 0)
                normalize(hB, qh, gav[hB], 0)
            if qh == 1 and i % 4 == 3 and i < 12:
                # interleave the jh=0 out-projection into the qh=1 stream
                outproj_m(0, 3 * mt + (i - 3) // 4)
            if i == kmax - 1:
                if qh == 1 and mt == 1:
                    # fill the PE gap while the last normalizes drain
                    outproj_m(0, 6)
                    outproj_m(0, 7)
                normalize(hA, qh, gav[hA], 1)
                normalize(hB, qh, gav[hB], 1)

        pending = None
        for h, qh, mt, i, gav in steps:
            pt = attn_se(h, qh, i)
            if pending is not None:
                ph, pqh, pmt, pi, pgav, ppt = pending
                attn_pv(ph, pqh, pi, pgav[ph], ppt)
                post_pv(ph, pqh, pmt, pi, pgav)
            if h not in gav:
                av_t = avp.tile([65, 2, 512], F32, tag="av")
                gav[h] = av_t
            pending = (h, qh, mt, i, gav, pt)
        ph, pqh, pmt, pi, pgav, ppt = pending
        attn_pv(ph, pqh, pi, pgav[ph], ppt)
        post_pv(ph, pqh, pmt, pi, pgav)

        # ---- phase 4: remaining out-projection (jh=1) ----
        for m in range(8):
            outproj_m(1, m)


class _pin_act_table:
    """Force every activation we use (Exp, Ln, Copy, Square) onto the one
    table set containing them all, so the program does a single
    ACT_TABLE_LOAD. Restores the shared cached dict on exit."""

    def __init__(self, arch):
        from concourse.hw_specs import get_activation_tables

        self.tabs = get_activation_tables(arch)

    def __enter__(self):
        self.saved = {nm: set(s) for nm, s in self.tabs.items()}
        for nm, s in self.tabs.items():
            if nm != "natural_log_exp_and_others":
                s.clear()

    def __exit__(self, *a):
        for nm, s in self.tabs.items():
            s.clear()
            s.update(self.saved[nm])


def build_program(iters=1):
    nc = bacc.Bacc(
        "TRN2",
        target_bir_lowering=False,
        debug=False,
        enable_asserts=False,
        num_devices=NCORES,
    )
    with tile.TileContext(nc) as tc:
        io = _declare_io(nc)
        for it in range(iters):
            _emit(tc, io, u=f"_i{it}" if iters > 1 else "")
    with _pin_act_table(nc.m.arch):
        nc.compile()
    return nc


def make_core_inputs(x, qkv_w, out_w, qn_w, kn_w, rope_cos, rope_sin, attention_mask):
    """Host-side shard/layout prep. Returns list of 8 per-core input dicts."""
    x = np.asarray(x, np.float32)
    qkv_w = np.asarray(qkv_w, np.float32)
    out_w = np.asarray(out_w, np.float32)
    qn_w = np.asarray(qn_w, np.float32)
    kn_w = np.asarray(kn_w, np.float32)
    rope_cos = np.asarray(rope_cos, np.float32)
    rope_sin = np.asarray(rope_sin, np.float32)
    am = np.asarray(attention_mask)

    r = qkv_w.reshape(3, H, HD, D)
    csT = rope_cos.T.astype(np.float32)                # (64, S)
    snT = rope_sin.T.astype(np.float32)
    s2 = np.concatenate([-snT[0:32], snT[32:64]], axis=0)  # sign-folded sin
    perm = np.concatenate([np.arange(32, 64), np.arange(0, 32)])

    def fold(tab, w, permute):
        ww = w[perm] if permute else w
        return (tab * ww[:, None]).astype(BF16)        # (64, S)

    csq = fold(csT, qn_w, False)
    snq = fold(s2, qn_w, True)
    csk = fold(csT, kn_w, False)
    snk = fold(s2, kn_w, True)

    # rank-8 factorization of the (128,128) diagonal-block mask
    dis = ~(am[0:128, 0:128].T)                        # dis[k', q'] disallowed
    mu = np.zeros((8, 128), np.float32)
    mv = np.zeros((8, 128), np.float32)
    for t in range(8):
        mu[t] = np.arange(128) // 16 == t
        mv[t] = -MASK_C * dis[16 * t, :]
    ones2 = np.zeros((128, 2), np.float32)
    ones2[0:64, 0] = 1.0
    ones2[64:128, 1] = 1.0
    b0 = float(HD * SCALE * max(1e-30, np.abs(qn_w).max() * np.abs(kn_w).max()))
    b0_t = np.full((128, 1), -b0, np.float32)

    shared = dict(
        csq=csq,
        snq=snq,
        csk=csk,
        snk=snk,
        mu=mu.astype(BF16),
        mv=mv.astype(BF16),
        ones2=ones2.astype(BF16),
        b0=b0_t,
    )
    in_maps = []
    for c in range(NCORES):
        b, g = divmod(c, 4)
        hs = slice(HLOC * g, HLOC * (g + 1))
        m = dict(shared)
        m["xt"] = np.ascontiguousarray(x[b].T).astype(BF16)

        def _wlayout(w):
            # (D, M) -> (128, NDK*M): partition p holds [t, m] = w[t*128+p, m]
            mm = w.shape[1]
            return np.ascontiguousarray(
                w.reshape(-1, 128, mm).transpose(1, 0, 2).reshape(128, -1)
            ).astype(BF16)

        m["wq"] = _wlayout(r[0, hs].transpose(2, 0, 1).reshape(D, 256))
        m["wk"] = _wlayout(r[1, hs].transpose(2, 0, 1).reshape(D, 256))
        m["wv"] = _wlayout(r[2, hs].transpose(2, 0, 1).reshape(D, 256))
        m["wo"] = _wlayout(
            np.ascontiguousarray(out_w[:, 256 * g : 256 * (g + 1)].T)
        )
        in_maps.append(m)
    return in_maps


_PROGRAM = []


def get_program():
    if not _PROGRAM:
        _PROGRAM.append(build_program())
    return _PROGRAM[0]


def unshard(results):
    """results: list of 8 dicts with 'yt' (32, 128, 512) f16 partials."""
    ys = []
    for b in range(B):
        acc = np.zeros((32, 128, 512), np.float32)
        for g in range(4):
            acc += np.asarray(results[4 * b + g]["yt"], np.float32)
        yt = acc.reshape(8, 4, 128, 512).transpose(0, 2, 1, 3).reshape(D, S)
        ys.append(yt.T.astype(np.float32))
    return np.stack(ys)


def kernel(**inputs):
    in_maps = make_core_inputs(**inputs)
    nc = get_program()
    res = run_bass_kernel_spmd(nc, in_maps, core_ids=list(range(NCORES)))
    return unshard(res.results)


# revision 4
# speedup vs baseline: 1.2167x; 1.2167x over previous
"""Block-causal attention (B=2, S=2048, D=1024, H=16, HD=64, BLOCK=16) on 8 TRN2 cores.

Sharding: core c -> batch c//4, head-group c%4 (4 heads). Each core computes the
full attention for its 4 heads plus a partial out-projection y^T (1024, 2048) in
f16; the host sums the 4 partials per batch and transposes.

v2 restructure vs baseline:
  - xt DMA in column blocks (arrival order == consumption order) so the Q0
    projection starts ~2us in.
  - proj order Q0,K0 -> lnexp group0 -> V -> Q1,K1 -> lnexp group1. RMS-norm
    Ln/Exp merged per group: pairs live at 32-partition offsets in one psum
    tile, one Ln + one Exp on [34, 2048] instead of 4 instructions apiece.
  - squares on ACT (Square), psum->sbuf raw copies split DVE/ACT to balance
    engines under the PE roofline.
  - attention qh-outer with the two heads of an mt pair interleaved per
    k-tile: their K=64 score matmuls sit at tile_position (0,0)/(64,0) and
    run concurrently on HW; second mask-factor copy lives at partitions 64-71.
  - av evacuated to SBUF by DVE right after the last PV so the psum bank
    frees early; normalize runs from the SBUF copy.
  - out-proj jh=0 interleaved into the qh=1 attention stream (shared score
    psum pool); output yt in f16 (host sums partials in f32).
"""

import numpy as np
import ml_dtypes

import concourse.bass as bass
import concourse.tile as tile
from concourse import bacc
from concourse import mybir
from concourse.bass_utils import run_bass_kernel_spmd

BF16 = ml_dtypes.bfloat16
F32 = mybir.dt.float32
F16 = mybir.dt.float16
BF = mybir.dt.bfloat16

B, S, D, H, HD = 2, 2048, 1024, 16, 64
HLOC = 4          # heads per core
NCORES = 8
EPS = 1e-6
SCALE = HD ** -0.5
MASK_C = 8192.0   # masked-pair score offset; exp underflows to 0.0
NST = 4           # 512-wide seq tiles
NKT = 16          # 128-wide key tiles
NDK = 8           # 128-wide model-dim tiles


def _declare_io(nc):
    def din(name, shape, d=BF):
        return nc.dram_tensor(name, shape, d, kind="ExternalInput").ap()

    io = dict(
        xt_d=din("xt", [D, S]),
        wq_d=din("wq", [128, NDK * 256]),
        wk_d=din("wk", [128, NDK * 256]),
        wv_d=din("wv", [128, NDK * 256]),
        wo_d=din("wo", [128, 2 * D]),
        csq_d=din("csq", [64, S]),
        snq_d=din("snq", [64, S]),
        csk_d=din("csk", [64, S]),
        snk_d=din("snk", [64, S]),
        mu_d=din("mu", [8, 128]),
        mv_d=din("mv", [8, 128]),
        ones2_d=din("ones2", [128, 2]),
        b0_d=din("b0", [128, 1], F32),
        yt_d=nc.dram_tensor(
            "yt", [32, 128, 512], F16, kind="ExternalOutput"
        ).ap(),
    )
    return io


def _emit(tc, io, u=""):
    """Emit the per-core program. Pure SPMD: identical on all 8 cores."""
    from contextlib import ExitStack

    nc = tc.nc
    A = mybir.ActivationFunctionType
    xt_d = io["xt_d"]
    wo_d = io["wo_d"]
    mu_d = io["mu_d"]
    mv_d = io["mv_d"]
    ones2_d = io["ones2_d"]
    b0_d = io["b0_d"]
    yt_d = io["yt_d"]

    ctx = ExitStack()
    proj_ctx = ExitStack()
    with ctx:
        consts = ctx.enter_context(tc.tile_pool(name="consts" + u, bufs=1))
        persist = ctx.enter_context(tc.tile_pool(name="persist" + u, bufs=1))
        dscratch = ctx.enter_context(
            tc.tile_pool(name="dscratch" + u, bufs=1, space="DRAM")
        )
        xtp = proj_ctx.enter_context(tc.tile_pool(name="xtp" + u, bufs=1))
        work2 = proj_ctx.enter_context(tc.tile_pool(name="work2" + u, bufs=2))
        sqp = proj_ctx.enter_context(tc.tile_pool(name="sqp" + u, bufs=3))
        pp = proj_ctx.enter_context(
            tc.tile_pool(name="pp" + u, bufs=2, space="PSUM")
        )
        msp = proj_ctx.enter_context(
            tc.tile_pool(name="msp" + u, bufs=2, space="PSUM")
        )

        # ---- input DMA: xt in column blocks (2 chunks per 512-col block)
        # on the sync/scalar HWDGE queues; weights+tables on the gpsimd
        # SWDGE queue in consumption order ----
        xt_sb = xtp.tile([128, NDK, S], BF)
        xt_rd = xt_d.rearrange("(a p) c -> p a c", p=128)
        for st in range(NST):
            sl = slice(512 * st, 512 * (st + 1))
            nc.sync.dma_start(out=xt_sb[:, 0:4, sl], in_=xt_rd[:, 0:4, sl])
            nc.scalar.dma_start(out=xt_sb[:, 4:8, sl], in_=xt_rd[:, 4:8, sl])

        wq_sb = consts.tile([128, NDK, 256], BF)
        wk_sb = consts.tile([128, NDK, 256], BF)
        wv_sb = consts.tile([128, NDK, 256], BF)
        wo_sb = consts.tile([128, 2, D], BF)
        for nm, t in (("wv", wv_sb), ("wq", wq_sb), ("wk", wk_sb)):
            nc.gpsimd.dma_start(
                out=t, in_=io[nm + "_d"].rearrange("p (t m) -> p t m", t=NDK)
            )
        # rope tables: DRAM holds 64 rows; duplicate into both SBUF halves
        csq_sb = consts.tile([128, S], BF)
        snq_sb = consts.tile([128, S], BF)
        csk_sb = consts.tile([128, S], BF)
        snk_sb = consts.tile([128, S], BF)
        for nm, t in (
            ("csq", csq_sb), ("snq", snq_sb), ("csk", csk_sb), ("snk", snk_sb)
        ):
            nc.gpsimd.dma_start(out=t[0:64], in_=io[nm + "_d"])
            nc.gpsimd.dma_start(out=t[64:128], in_=io[nm + "_d"])
        nc.gpsimd.dma_start(out=wo_sb, in_=wo_d.rearrange("p (t m) -> p t m", t=2))
        # rank-8 mask factors at partitions 0-7 (heads at po=0) and a second
        # copy at partitions 64-71 (heads at po=64) for PE row-group overlap
        mu_sb = consts.tile([8, 128], BF)
        mv_sb = consts.tile([8, 128], BF)
        mm64 = consts.tile([72, 2, 128], BF)
        nc.sync.dma_start(out=mu_sb, in_=mu_d)
        nc.sync.dma_start(out=mv_sb, in_=mv_d)
        nc.sync.dma_start(out=mm64[64:72, 0, :], in_=mu_d)
        nc.sync.dma_start(out=mm64[64:72, 1, :], in_=mv_d)
        ones2_sb = consts.tile([128, 2], BF)
        nc.sync.dma_start(out=ones2_sb, in_=ones2_d)
        b0_sb = consts.tile([128, 1], F32)
        nc.sync.dma_start(out=b0_sb, in_=b0_d)
        eps_sb = consts.tile([128, 1], F32)
        nc.vector.memset(eps_sb, EPS)

        # ---- persistent activations ----
        qT = persist.tile([128, 2, S], BF)      # (2 heads)*64 rows per m-tile
        kT = persist.tile([128, 2, S], BF)
        vv = persist.tile([128, NKT, HLOC, HD + 1], BF)   # [V | ones]
        at = persist.tile([128, 2, S], BF)      # normalized attn^T
        # pair p's two rrms rows live at partition 32*p
        ln8 = persist.tile([98, NST, 512], F32)
        rr8 = persist.tile([98, NST, 512], BF)
        rkb = persist.tile([128, 64], BF)    # k-side rrms, (k mod 128, h*16+i)
        rkz = persist.tile([128, 4, 16], F32)  # SCALE * rrms_k per (head, ktile)
        rr_dram = dscratch.tile([8, 16, 128], BF)

        nc.vector.memset(vv[:, :, :, HD : HD + 1], 1.0)

        # ---- phase 1: Q/K projections + RMS-norm stats + RoPE ----
        def proj_pair(qk_i, mt):
            """Project pair (qk_i: 0=Q, 1=K) for m-tile mt; fill qraw, compute
            rrms = exp(-0.5*ln(ms/HD+eps)) into rr8. Returns the qraw tile."""
            pair = 2 * mt + qk_i
            pb = 32 * pair
            wsb = wq_sb if qk_i == 0 else wk_sb
            qraw = work2.tile([128, S], BF, tag="qraw")
            for hf in range(2):
                ms_t = msp.tile([2, 2, 512], F32, tag="ms")
                for s2 in range(2):
                    st = 2 * hf + s2
                    ps = pp.tile([128, 512], F32, tag="pp")
                    for kt in range(NDK):
                        nc.tensor.matmul(
                            ps,
                            lhsT=wsb[:, kt, 128 * mt : 128 * (mt + 1)],
                            rhs=xt_sb[:, kt, 512 * st : 512 * (st + 1)],
                            start=(kt == 0),
                            stop=(kt == NDK - 1),
                        )
                    sl = slice(512 * st, 512 * (st + 1))
                    if st % 2 == 0:
                        nc.vector.tensor_copy(qraw[:, sl], ps)
                    else:
                        nc.scalar.copy(qraw[:, sl], ps)
                    sq = sqp.tile([128, 512], BF, tag="sq")
                    nc.scalar.activation(sq, qraw[:, sl], A.Square)
                    nc.tensor.matmul(
                        ms_t[:, s2, :],
                        lhsT=ones2_sb,
                        rhs=sq,
                        start=True,
                        stop=True,
                    )
                nc.scalar.activation(
                    ln8[pb : pb + 2, 2 * hf : 2 * hf + 2, :],
                    ms_t,
                    A.Ln,
                    bias=eps_sb[0:2],
                    scale=1.0 / HD,
                )
                nc.scalar.activation(
                    rr8[pb : pb + 2, 2 * hf : 2 * hf + 2, :],
                    ln8[pb : pb + 2, 2 * hf : 2 * hf + 2, :],
                    A.Exp,
                    scale=-0.5,
                )
            return qraw

        def rope(qk_i, mt, qraw):
            """RoPE on raw projection output. Q side: multiply by the
            broadcast rrms afterwards; K side: plain add (rrms folded into
            the exp scale)."""
            pair = 2 * mt + qk_i
            pb = 32 * pair
            cstab = csq_sb if qk_i == 0 else csk_sb
            sntab = snq_sb if qk_i == 0 else snk_sb
            dest = qT if qk_i == 0 else kT
            rot = work2.tile([128, S], BF, tag="rot")
            for lo, hi in ((0, 32), (32, 64), (64, 96), (96, 128)):
                src_lo = lo + 32 if (lo // 32) % 2 == 0 else lo - 32
                eng = nc.sync if lo < 64 else nc.scalar
                eng.dma_start(out=rot[lo:hi], in_=qraw[src_lo : src_lo + 32])
            t1 = work2.tile([128, S], BF, tag="t1")
            t2 = work2.tile([128, S], BF, tag="t2")
            nc.vector.tensor_mul(t1, qraw, cstab)
            nc.vector.tensor_mul(t2, rot, sntab)
            nc.sync.dma_start(
                out=rr_dram[2 * pair : 2 * pair + 2].rearrange("r a b -> r (a b)"),
                in_=rr8[pb : pb + 2].rearrange("p a b -> p (a b)"),
            )
            if qk_i == 1:
                nc.vector.tensor_add(dest[:, mt, :], t1, t2)
                # k-side rrms rows -> partition-major via DMA transpose,
                # folding in the 1/sqrt(HD) softmax scale
                nc.sync.dma_start_transpose(
                    rkb[:, 32 * mt : 32 * (mt + 1)],
                    rr_dram[2 * pair : 2 * pair + 2].rearrange("r a b -> (r a) b"),
                )
                nc.vector.tensor_scalar_mul(
                    rkz[:, 2 * mt : 2 * mt + 2, :].rearrange("p h i -> p (h i)"),
                    rkb[:, 32 * mt : 32 * (mt + 1)],
                    SCALE,
                )
            else:
                tsum = work2.tile([128, S], BF, tag="tsum")
                nc.vector.tensor_add(tsum, t1, t2)
                rrb = work2.tile([128, NST, 512], BF, tag="rrb")
                nc.gpsimd.dma_start(
                    out=rrb[0:64],
                    in_=rr_dram[2 * pair : 2 * pair + 1]
                    .rearrange("r a b -> r (a b)")
                    .rearrange("r (a b) -> r a b", a=NST)
                    .partition_broadcast(64),
                )
                nc.gpsimd.dma_start(
                    out=rrb[64:128],
                    in_=rr_dram[2 * pair + 1 : 2 * pair + 2]
                    .rearrange("r a b -> r (a b)")
                    .rearrange("r (a b) -> r a b", a=NST)
                    .partition_broadcast(64),
                )
                for st in range(NST):
                    sl = slice(512 * st, 512 * (st + 1))
                    nc.vector.tensor_mul(
                        dest[:, mt, sl], tsum[:, sl], rrb[:, st, :]
                    )

        # ---- phase 1a: V projection first — it only needs xt, so it rides
        # the tail of the input DMA; two st tiles per psum bank ----
        for sp2 in range(NKT // 2):
            ps = pp.tile([128, 512], F32, tag="pp")
            for half in range(2):
                stv = 2 * sp2 + half
                for kt in range(NDK):
                    nc.tensor.matmul(
                        ps[:, 256 * half : 256 * (half + 1)],
                        lhsT=xt_sb[:, kt, 128 * stv : 128 * (stv + 1)],
                        rhs=wv_sb[:, kt, :],
                        start=(kt == 0),
                        stop=(kt == NDK - 1),
                    )
            nc.vector.tensor_copy(
                vv[:, 2 * sp2 : 2 * sp2 + 2, :, 0:HD],
                ps.rearrange("p (a h d) -> p a h d", a=2, h=HLOC),
            )

        for mt in range(2):
            qraw_q = proj_pair(0, mt)
            rope(0, mt, qraw_q)
            qraw_k = proj_pair(1, mt)
            rope(1, mt, qraw_k)

        # proj scratch (incl. x^T) is dead now; free SBUF/PSUM for attention
        proj_ctx.close()
        attnw = ctx.enter_context(tc.tile_pool(name="attnw" + u, bufs=3))
        ptp = ctx.enter_context(tc.tile_pool(name="ptp" + u, bufs=6))
        ystp = ctx.enter_context(tc.tile_pool(name="ystp" + u, bufs=3))
        spp = ctx.enter_context(tc.tile_pool(name="spp" + u, bufs=2, space="PSUM"))
        avp = ctx.enter_context(tc.tile_pool(name="avp" + u, bufs=2, space="PSUM"))

        def attn_se(h, qh, i):
            """Scores + exp for head h, query-half qh, k-tile i -> pt."""
            mt, half = divmod(h, 2)
            po = 64 * half
            glo = 1024 * qh
            q0 = 128 * i
            lo_g = max(glo, q0)
            pt = ptp.tile([128, 1024], BF, tag="pt")
            sp = spp.tile([128, 1024], F32, tag="spp")
            has_diag = glo <= q0 < glo + 1024
            for jj in range(2):
                j = 2 * qh + jj
                lo = max(512 * j, q0)
                hi = 512 * (j + 1)
                if lo >= hi:
                    continue
                diag_bank = has_diag and (q0 - glo) // 512 == jj
                nc.tensor.matmul(
                    sp[:, lo - glo : hi - glo],
                    lhsT=kT[po : po + 64, mt, 128 * i : 128 * (i + 1)],
                    rhs=qT[po : po + 64, mt, lo:hi],
                    start=True,
                    stop=not diag_bank,
                )
                if diag_bank:
                    # block-causal mask: scores -= MASK_C * disallowed
                    mum = mu_sb if half == 0 else mm64[64:72, 0, :]
                    mvm = mv_sb if half == 0 else mm64[64:72, 1, :]
                    nc.tensor.matmul(
                        sp[:, q0 - glo : q0 - glo + 128],
                        lhsT=mum,
                        rhs=mvm,
                        start=False,
                        stop=True,
                    )
            # P^T = exp(rrms_k[k]/sqrt(HD) * scores - B0)
            nc.scalar.activation(
                pt[:, lo_g - glo : 1024],
                sp[:, lo_g - glo : 1024],
                A.Exp,
                bias=b0_sb,
                scale=rkz[:, h, i : i + 1],
            )
            return pt

        def attn_pv(h, qh, i, av, pt):
            """attn^T accumulation (+ denominator in row 64)."""
            glo = 1024 * qh
            kmax = 8 * (qh + 1)
            q0 = 128 * i
            for jj in range(2):
                j = 2 * qh + jj
                jlo = max(512 * j, q0)
                jhi = 512 * (j + 1)
                if jlo >= jhi:
                    continue
                nc.tensor.matmul(
                    av[:, jj, jlo - 512 * j : 512],
                    lhsT=vv[:, i, h, :],
                    rhs=pt[:, jlo - glo : jhi - glo],
                    start=(i == 0),
                    stop=(i == min(kmax, 4 * j + 4) - 1),
                )

        def normalize(h, qh, av, jj):
            """Evacuate one jj half of av to SBUF (its accumulation closed at
            i=4j+3), then divide the 64 head rows by the denominator row.
            Splitting by jj lets half the work run inside the i loop and
            frees the psum bank sooner at group end."""
            mt, half = divmod(h, 2)
            po = 64 * half
            lo = 1024 * qh + 512 * jj
            avs = attnw.tile([65, 512], F32, tag="avs")
            nc.vector.tensor_copy(avs, av[:, jj, :])
            rden = attnw.tile([1, 512], F32, tag="rden")
            nc.vector.reciprocal(rden, avs[64:65])
            rdb = attnw.tile([64, 512], F32, tag="rdb")
            nc.gpsimd.partition_broadcast(rdb, rden, channels=64)
            nc.vector.tensor_mul(
                at[po : po + 64, mt, lo : lo + 512],
                avs[0:64],
                rdb,
            )

        def outproj_m(jh, m):
            """One 128-row tile of the partial out-projection y^T."""
            ps = spp.tile([128, 1024], F32, tag="spp")
            for jj in range(2):
                j = 2 * jh + jj
                for kt in range(2):
                    nc.tensor.matmul(
                        ps[:, 512 * jj : 512 * (jj + 1)],
                        lhsT=wo_sb[:, kt, 128 * m : 128 * (m + 1)],
                        rhs=at[:, kt, 512 * j : 512 * (j + 1)],
                        start=(kt == 0),
                        stop=(kt == 1),
                    )
            yst = ystp.tile([128, 2, 512], F16, tag="yst")
            # jh=0 runs inside the attention window where ACT is the
            # bottleneck -> DVE; jh=1 is the tail where ACT idles -> mostly ACT
            # (DVE carries the trailing normalizes there).
            if jh == 1 and m % 2 == 1:
                nc.scalar.copy(yst, ps.rearrange("p (a b) -> p a b", a=2))
            else:
                nc.vector.tensor_copy(yst, ps.rearrange("p (a b) -> p a b", a=2))
            eng = nc.sync if m % 2 == 0 else nc.scalar
            eng.dma_start(
                out=yt_d[4 * m + 2 * jh : 4 * m + 2 * jh + 2].rearrange(
                    "a p b -> p a b"
                ),
                in_=yst,
            )

        # ---- phase 3: attention, qh-outer, head pairs interleaved, with a
        # one-step software pipeline: scores+exp of step N issue before the
        # PV of step N-1, so group boundaries never starve the ACT engine ----
        steps = []
        for qh in range(2):
            for mt in range(2):
                gav = {}
                for i in range(8 * (qh + 1)):
                    for h in (2 * mt, 2 * mt + 1):
                        steps.append((h, qh, mt, i, gav))

        def post_pv(h, qh, mt, i, gav):
            """Hooks that must follow the PV of (h, qh, i)."""
            hA, hB = 2 * mt, 2 * mt + 1
            if h != hB:
                return
            kmax = 8 * (qh + 1)
            jj0_done = 3 if qh == 0 else 11
            if i == jj0_done:
                # first query-half of av closed -> normalize it now
                normalize(hA, qh, gav[hA], 0)
                normalize(hB, qh, gav[hB], 0)
            if qh == 1 and i % 4 == 3 and i < 12:
                # interleave the jh=0 out-projection into the qh=1 stream
                outproj_m(0, 3 * mt + (i - 3) // 4)
            if i == kmax - 1:
                if qh == 1 and mt == 1:
                    # fill the PE gap while the last normalizes drain
                    outproj_m(0, 6)
                    outproj_m(0, 7)
                normalize(hA, qh, gav[hA], 1)
                normalize(hB, qh, gav[hB], 1)

        pending = None
        for h, qh, mt, i, gav in steps:
            pt = attn_se(h, qh, i)
            if pending is not None:
                ph, pqh, pmt, pi, pgav, ppt = pending
                attn_pv(ph, pqh, pi, pgav[ph], ppt)
                post_pv(ph, pqh, pmt, pi, pgav)
            if h not in gav:
                av_t = avp.tile([65, 2, 512], F32, tag="av")
                gav[h] = av_t
            pending = (h, qh, mt, i, gav, pt)
        ph, pqh, pmt, pi, pgav, ppt = pending
        attn_pv(ph, pqh, pi, pgav[ph], ppt)
        post_pv(ph, pqh, pmt, pi, pgav)

        # ---- phase 4: remaining out-projection (jh=1) ----
        for m in range(8):
            outproj_m(1, m)


class _pin_act_table:
    """Force every activation we use (Exp, Ln, Copy, Square) onto the one
    table set containing them all, so the program does a single
    ACT_TABLE_LOAD. Restores the shared cached dict on exit."""

    def __init__(self, arch):
        from concourse.hw_specs import get_activation_tables

        self.tabs = get_activation_tables(arch)

    def __enter__(self):
        self.saved = {nm: set(s) for nm, s in self.tabs.items()}
        for nm, s in self.tabs.items():
            if nm != "natural_log_exp_and_others":
                s.clear()

    def __exit__(self, *a):
        for nm, s in self.tabs.items():
            s.clear()
            s.update(self.saved[nm])


def build_program(iters=1):
    nc = bacc.Bacc(
        "TRN2",
        target_bir_lowering=False,
        debug=False,
        enable_asserts=False,
        num_devices=NCORES,
    )
    with tile.TileContext(nc) as tc:
        io = _declare_io(nc)
        for it in range(iters):
            _emit(tc, io, u=f"_i{it}" if iters > 1 else "")
    with _pin_act_table(nc.m.arch):
        nc.compile()
    return nc


def make_core_inputs(x, qkv_w, out_w, qn_w, kn_w, rope_cos, rope_sin, attention_mask):
    """Host-side shard/layout prep. Returns list of 8 per-core input dicts."""
    x = np.asarray(x, np.float32)
    qkv_w = np.asarray(qkv_w, np.float32)
    out_w = np.asarray(out_w, np.float32)
    qn_w = np.asarray(qn_w, np.float32)
    kn_w = np.asarray(kn_w, np.float32)
    rope_cos = np.asarray(rope_cos, np.float32)
    rope_sin = np.asarray(rope_sin, np.float32)
    am = np.asarray(attention_mask)

    r = qkv_w.reshape(3, H, HD, D)
    csT = rope_cos.T.astype(np.float32)                # (64, S)
    snT = rope_sin.T.astype(np.float32)
    s2 = np.concatenate([-snT[0:32], snT[32:64]], axis=0)  # sign-folded sin
    perm = np.concatenate([np.arange(32, 64), np.arange(0, 32)])

    def fold(tab, w, permute):
        ww = w[perm] if permute else w
        return (tab * ww[:, None]).astype(BF16)        # (64, S)

    csq = fold(csT, qn_w, False)
    snq = fold(s2, qn_w, True)
    csk = fold(csT, kn_w, False)
    snk = fold(s2, kn_w, True)

    # rank-8 factorization of the (128,128) diagonal-block mask
    dis = ~(am[0:128, 0:128].T)                        # dis[k', q'] disallowed
    mu = np.zeros((8, 128), np.float32)
    mv = np.zeros((8, 128), np.float32)
    for t in range(8):
        mu[t] = np.arange(128) // 16 == t
        mv[t] = -MASK_C * dis[16 * t, :]
    ones2 = np.zeros((128, 2), np.float32)
    ones2[0:64, 0] = 1.0
    ones2[64:128, 1] = 1.0
    b0 = float(HD * SCALE * max(1e-30, np.abs(qn_w).max() * np.abs(kn_w).max()))
    b0_t = np.full((128, 1), -b0, np.float32)

    shared = dict(
        csq=csq,
        snq=snq,
        csk=csk,
        snk=snk,
        mu=mu.astype(BF16),
        mv=mv.astype(BF16),
        ones2=ones2.astype(BF16),
        b0=b0_t,
    )
    in_maps = []
    for c in range(NCORES):
        b, g = divmod(c, 4)
        hs = slice(HLOC * g, HLOC * (g + 1))
        m = dict(shared)
        m["xt"] = np.ascontiguousarray(x[b].T).astype(BF16)

        def _wlayout(w):
            # (D, M) -> (128, NDK*M): partition p holds [t, m] = w[t*128+p, m]
            mm = w.shape[1]
            return np.ascontiguousarray(
                w.reshape(-1, 128, mm).transpose(1, 0, 2).reshape(128, -1)
            ).astype(BF16)

        m["wq"] = _wlayout(r[0, hs].transpose(2, 0, 1).reshape(D, 256))
        m["wk"] = _wlayout(r[1, hs].transpose(2, 0, 1).reshape(D, 256))
        m["wv"] = _wlayout(r[2, hs].transpose(2, 0, 1).reshape(D, 256))
        m["wo"] = _wlayout(
            np.ascontiguousarray(out_w[:, 256 * g : 256 * (g + 1)].T)
        )
        in_maps.append(m)
    return in_maps


_PROGRAM = []


def get_program():
    if not _PROGRAM:
        _PROGRAM.append(build_program())
    return _PROGRAM[0]


def unshard(results):
    """results: list of 8 dicts with 'yt' (32, 128, 512) f16 partials."""
    ys = []
    for b in range(B):
        acc = np.zeros((32, 128, 512), np.float32)
        for g in range(4):
            acc += np.asarray(results[4 * b + g]["yt"], np.float32)
        yt = acc.reshape(8, 4, 128, 512).transpose(0, 2, 1, 3).reshape(D, S)
        ys.append(yt.T.astype(np.float32))
    return np.stack(ys)


def kernel(**inputs):
    in_maps = make_core_inputs(**inputs)
    nc = get_program()
    res = run_bass_kernel_spmd(nc, in_maps, core_ids=list(range(NCORES)))
    return unshard(res.results)


# revision 25
# speedup vs baseline: 1.3825x; 1.1363x over previous
"""Block-causal attention (B=2, S=2048, D=1024, H=16, HD=64, BLOCK=16) on 8 TRN2 cores.

Sharding: core c -> batch c//4, head-group c%4 (4 heads). Each core computes the
full attention for its 4 heads plus a partial out-projection y^T (1024, 2048) in
f16; the host sums the 4 partials per batch and transposes.

v2 restructure vs baseline:
  - xt DMA in column blocks (arrival order == consumption order) so the Q0
    projection starts ~2us in.
  - proj order Q0,K0 -> lnexp group0 -> V -> Q1,K1 -> lnexp group1. RMS-norm
    Ln/Exp merged per group: pairs live at 32-partition offsets in one psum
    tile, one Ln + one Exp on [34, 2048] instead of 4 instructions apiece.
  - squares on ACT (Square), psum->sbuf raw copies split DVE/ACT to balance
    engines under the PE roofline.
  - attention qh-outer with the two heads of an mt pair interleaved per
    k-tile: their K=64 score matmuls sit at tile_position (0,0)/(64,0) and
    run concurrently on HW; second mask-factor copy lives at partitions 64-71.
  - av evacuated to SBUF by DVE right after the last PV so the psum bank
    frees early; normalize runs from the SBUF copy.
  - out-proj jh=0 interleaved into the qh=1 attention stream (shared score
    psum pool); output yt in f16 (host sums partials in f32).
"""

import numpy as np
import ml_dtypes

import concourse.bass as bass
import concourse.tile as tile
from concourse import bacc
from concourse import mybir
from concourse.bass_utils import run_bass_kernel_spmd

BF16 = ml_dtypes.bfloat16
F32 = mybir.dt.float32
F16 = mybir.dt.float16
BF = mybir.dt.bfloat16

B, S, D, H, HD = 2, 2048, 1024, 16, 64
HLOC = 4          # heads per core
NCORES = 8
EPS = 1e-6
SCALE = HD ** -0.5
MASK_C = 8192.0   # masked-pair score offset; exp underflows to 0.0
NST = 4           # 512-wide seq tiles
NKT = 16          # 128-wide key tiles
NDK = 8           # 128-wide model-dim tiles


def _declare_io(nc):
    def din(name, shape, d=BF):
        return nc.dram_tensor(name, shape, d, kind="ExternalInput").ap()

    io = dict(
        xt_d=din("xt", [D, S]),
        wq_d=din("wq", [128, NDK * 256]),
        wk_d=din("wk", [128, NDK * 256]),
        wv_d=din("wv", [128, NDK * 256]),
        wo_d=din("wo", [128, 2 * D]),
        csq_d=din("csq", [64, S]),
        snq_d=din("snq", [64, S]),
        csk_d=din("csk", [64, S]),
        snk_d=din("snk", [64, S]),
        mu_d=din("mu", [8, 128]),
        mv_d=din("mv", [8, 128]),
        ones2_d=din("ones2", [128, 2]),
        b0_d=din("b0", [128, 1], F32),
        yt_d=nc.dram_tensor(
            "yt", [32, 128, 512], F16, kind="ExternalOutput"
        ).ap(),
    )
    return io


def _emit(tc, io, u=""):
    """Emit the per-core program. Pure SPMD: identical on all 8 cores."""
    from contextlib import ExitStack

    nc = tc.nc
    A = mybir.ActivationFunctionType
    xt_d = io["xt_d"]
    wo_d = io["wo_d"]
    mu_d = io["mu_d"]
    mv_d = io["mv_d"]
    ones2_d = io["ones2_d"]
    b0_d = io["b0_d"]
    yt_d = io["yt_d"]

    ctx = ExitStack()
    proj_ctx = ExitStack()
    with ctx:
        consts = ctx.enter_context(tc.tile_pool(name="consts" + u, bufs=1))
        persist = ctx.enter_context(tc.tile_pool(name="persist" + u, bufs=1))
        dscratch = ctx.enter_context(
            tc.tile_pool(name="dscratch" + u, bufs=1, space="DRAM")
        )
        xtp = proj_ctx.enter_context(tc.tile_pool(name="xtp" + u, bufs=1))
        work2 = proj_ctx.enter_context(tc.tile_pool(name="work2" + u, bufs=2))
        sqp = proj_ctx.enter_context(tc.tile_pool(name="sqp" + u, bufs=3))
        pp = proj_ctx.enter_context(
            tc.tile_pool(name="pp" + u, bufs=2, space="PSUM")
        )
        msp = proj_ctx.enter_context(
            tc.tile_pool(name="msp" + u, bufs=2, space="PSUM")
        )

        # ---- input DMA: xt in column blocks (2 chunks per 512-col block)
        # on the sync/scalar HWDGE queues; weights+tables on the gpsimd
        # SWDGE queue in consumption order ----
        xt_sb = xtp.tile([128, NDK, S], BF)
        xt_rd = xt_d.rearrange("(a p) c -> p a c", p=128)
        for st in range(NST):
            sl = slice(512 * st, 512 * (st + 1))
            nc.sync.dma_start(out=xt_sb[:, 0:4, sl], in_=xt_rd[:, 0:4, sl])
            nc.scalar.dma_start(out=xt_sb[:, 4:8, sl], in_=xt_rd[:, 4:8, sl])

        wq_sb = consts.tile([128, NDK, 256], BF)
        wk_sb = consts.tile([128, NDK, 256], BF)
        wv_sb = consts.tile([128, NDK, 256], BF)
        wo_sb = consts.tile([128, 2, D], BF)
        for nm, t in (("wv", wv_sb), ("wq", wq_sb), ("wk", wk_sb)):
            nc.gpsimd.dma_start(
                out=t, in_=io[nm + "_d"].rearrange("p (t m) -> p t m", t=NDK)
            )
        # rope tables: DRAM holds 64 rows; duplicate into both SBUF halves
        csq_sb = consts.tile([128, S], BF)
        snq_sb = consts.tile([128, S], BF)
        csk_sb = consts.tile([128, S], BF)
        snk_sb = consts.tile([128, S], BF)
        for nm, t in (
            ("csq", csq_sb), ("snq", snq_sb), ("csk", csk_sb), ("snk", snk_sb)
        ):
            nc.gpsimd.dma_start(out=t[0:64], in_=io[nm + "_d"])
            nc.gpsimd.dma_start(out=t[64:128], in_=io[nm + "_d"])
        nc.gpsimd.dma_start(out=wo_sb, in_=wo_d.rearrange("p (t m) -> p t m", t=2))
        # rank-8 mask factors at partitions 0-7 (heads at po=0) and a second
        # copy at partitions 64-71 (heads at po=64) for PE row-group overlap
        mu_sb = consts.tile([8, 128], BF)
        mv_sb = consts.tile([8, 128], BF)
        mm64 = consts.tile([72, 2, 128], BF)
        nc.sync.dma_start(out=mu_sb, in_=mu_d)
        nc.sync.dma_start(out=mv_sb, in_=mv_d)
        nc.sync.dma_start(out=mm64[64:72, 0, :], in_=mu_d)
        nc.sync.dma_start(out=mm64[64:72, 1, :], in_=mv_d)
        ones2_sb = consts.tile([128, 2], BF)
        nc.sync.dma_start(out=ones2_sb, in_=ones2_d)
        b0_sb = consts.tile([128, 1], F32)
        nc.sync.dma_start(out=b0_sb, in_=b0_d)
        eps_sb = consts.tile([128, 1], F32)
        nc.vector.memset(eps_sb, EPS)

        # ---- persistent activations ----
        qT = persist.tile([128, 2, S], BF)      # (2 heads)*64 rows per m-tile
        kT = persist.tile([128, 2, S], BF)
        vv = persist.tile([128, NKT, HLOC, HD + 1], BF)   # [V | ones]
        at = persist.tile([128, 2, S], BF)      # normalized attn^T
        # pair p's two rrms rows live at partition 32*p
        ln8 = persist.tile([98, NST, 512], F32)
        rr8 = persist.tile([98, NST, 512], BF)
        rkb = persist.tile([128, 64], BF)    # k-side rrms, (k mod 128, h*16+i)
        rkz = persist.tile([128, 4, 16], F32)  # SCALE * rrms_k per (head, ktile)
        rr_dram = dscratch.tile([8, 16, 128], BF)

        nc.vector.memset(vv[:, :, :, HD : HD + 1], 1.0)

        # ---- phase 1: Q/K projections + RMS-norm stats + RoPE ----
        def proj_pair(qk_i, mt):
            """Project pair (qk_i: 0=Q, 1=K) for m-tile mt; fill qraw, compute
            rrms = exp(-0.5*ln(ms/HD+eps)) into rr8. Returns the qraw tile."""
            pair = 2 * mt + qk_i
            pb = 32 * pair
            wsb = wq_sb if qk_i == 0 else wk_sb
            qraw = work2.tile([128, S], BF, tag="qraw")
            for hf in range(2):
                ms_t = msp.tile([2, 2, 512], F32, tag="ms")
                for s2 in range(2):
                    st = 2 * hf + s2
                    ps = pp.tile([128, 512], F32, tag="pp")
                    for kt in range(NDK):
                        nc.tensor.matmul(
                            ps,
                            lhsT=wsb[:, kt, 128 * mt : 128 * (mt + 1)],
                            rhs=xt_sb[:, kt, 512 * st : 512 * (st + 1)],
                            start=(kt == 0),
                            stop=(kt == NDK - 1),
                        )
                    sl = slice(512 * st, 512 * (st + 1))
                    if st % 2 == 0:
                        nc.vector.tensor_copy(qraw[:, sl], ps)
                    else:
                        nc.scalar.copy(qraw[:, sl], ps)
                    sq = sqp.tile([128, 512], BF, tag="sq")
                    nc.scalar.activation(sq, qraw[:, sl], A.Square)
                    nc.tensor.matmul(
                        ms_t[:, s2, :],
                        lhsT=ones2_sb,
                        rhs=sq,
                        start=True,
                        stop=True,
                    )
                nc.scalar.activation(
                    ln8[pb : pb + 2, 2 * hf : 2 * hf + 2, :],
                    ms_t,
                    A.Ln,
                    bias=eps_sb[0:2],
                    scale=1.0 / HD,
                )
                nc.scalar.activation(
                    rr8[pb : pb + 2, 2 * hf : 2 * hf + 2, :],
                    ln8[pb : pb + 2, 2 * hf : 2 * hf + 2, :],
                    A.Exp,
                    scale=-0.5,
                )
            return qraw

        def rope(qk_i, mt, qraw):
            """RoPE on raw projection output. Q side: multiply by the
            broadcast rrms afterwards; K side: plain add (rrms folded into
            the exp scale)."""
            pair = 2 * mt + qk_i
            pb = 32 * pair
            cstab = csq_sb if qk_i == 0 else csk_sb
            sntab = snq_sb if qk_i == 0 else snk_sb
            dest = qT if qk_i == 0 else kT
            rot = work2.tile([128, S], BF, tag="rot")
            for lo, hi in ((0, 32), (32, 64), (64, 96), (96, 128)):
                src_lo = lo + 32 if (lo // 32) % 2 == 0 else lo - 32
                eng = nc.sync if lo < 64 else nc.scalar
                eng.dma_start(out=rot[lo:hi], in_=qraw[src_lo : src_lo + 32])
            t1 = work2.tile([128, S], BF, tag="t1")
            t2 = work2.tile([128, S], BF, tag="t2")
            nc.vector.tensor_mul(t1, qraw, cstab)
            nc.vector.tensor_mul(t2, rot, sntab)
            nc.sync.dma_start(
                out=rr_dram[2 * pair : 2 * pair + 2].rearrange("r a b -> r (a b)"),
                in_=rr8[pb : pb + 2].rearrange("p a b -> p (a b)"),
            )
            if qk_i == 1:
                nc.vector.tensor_add(dest[:, mt, :], t1, t2)
                # k-side rrms rows -> partition-major via DMA transpose,
                # folding in the 1/sqrt(HD) softmax scale
                nc.sync.dma_start_transpose(
                    rkb[:, 32 * mt : 32 * (mt + 1)],
                    rr_dram[2 * pair : 2 * pair + 2].rearrange("r a b -> (r a) b"),
                )
                nc.vector.tensor_scalar_mul(
                    rkz[:, 2 * mt : 2 * mt + 2, :].rearrange("p h i -> p (h i)"),
                    rkb[:, 32 * mt : 32 * (mt + 1)],
                    SCALE,
                )
            else:
                tsum = work2.tile([128, S], BF, tag="tsum")
                nc.vector.tensor_add(tsum, t1, t2)
                rrb = work2.tile([128, NST, 512], BF, tag="rrb")
                nc.gpsimd.dma_start(
                    out=rrb[0:64],
                    in_=rr_dram[2 * pair : 2 * pair + 1]
                    .rearrange("r a b -> r (a b)")
                    .rearrange("r (a b) -> r a b", a=NST)
                    .partition_broadcast(64),
                )
                nc.gpsimd.dma_start(
                    out=rrb[64:128],
                    in_=rr_dram[2 * pair + 1 : 2 * pair + 2]
                    .rearrange("r a b -> r (a b)")
                    .rearrange("r (a b) -> r a b", a=NST)
                    .partition_broadcast(64),
                )
                for st in range(NST):
                    sl = slice(512 * st, 512 * (st + 1))
                    nc.vector.tensor_mul(
                        dest[:, mt, sl], tsum[:, sl], rrb[:, st, :]
                    )

        # ---- phase 1a: V projection first — it only needs xt, so it rides
        # the tail of the input DMA; two st tiles per psum bank ----
        for sp2 in range(NKT // 2):
            ps = pp.tile([128, 512], F32, tag="pp")
            for half in range(2):
                stv = 2 * sp2 + half
                for kt in range(NDK):
                    nc.tensor.matmul(
                        ps[:, 256 * half : 256 * (half + 1)],
                        lhsT=xt_sb[:, kt, 128 * stv : 128 * (stv + 1)],
                        rhs=wv_sb[:, kt, :],
                        start=(kt == 0),
                        stop=(kt == NDK - 1),
                    )
            nc.vector.tensor_copy(
                vv[:, 2 * sp2 : 2 * sp2 + 2, :, 0:HD],
                ps.rearrange("p (a h d) -> p a h d", a=2, h=HLOC),
            )

        for mt in range(2):
            qraw_q = proj_pair(0, mt)
            rope(0, mt, qraw_q)
            qraw_k = proj_pair(1, mt)
            rope(1, mt, qraw_k)

        # proj scratch (incl. x^T) is dead now; free SBUF/PSUM for attention
        proj_ctx.close()
        attnw = ctx.enter_context(tc.tile_pool(name="attnw" + u, bufs=3))
        ptp = ctx.enter_context(tc.tile_pool(name="ptp" + u, bufs=6))
        ystp = ctx.enter_context(tc.tile_pool(name="ystp" + u, bufs=4))
        spp = ctx.enter_context(tc.tile_pool(name="spp" + u, bufs=2, space="PSUM"))
        avp = ctx.enter_context(tc.tile_pool(name="avp" + u, bufs=2, space="PSUM"))

        def attn_se(h, qh, i):
            """Scores + exp for head h, query-half qh, k-tile i -> pt."""
            mt, half = divmod(h, 2)
            po = 64 * half
            glo = 1024 * qh
            q0 = 128 * i
            lo_g = max(glo, q0)
            pt = ptp.tile([128, 1024], BF, tag="pt")
            sp = spp.tile([128, 1024], F32, tag="spp")
            has_diag = glo <= q0 < glo + 1024
            for jj in range(2):
                j = 2 * qh + jj
                lo = max(512 * j, q0)
                hi = 512 * (j + 1)
                if lo >= hi:
                    continue
                diag_bank = has_diag and (q0 - glo) // 512 == jj
                nc.tensor.matmul(
                    sp[:, lo - glo : hi - glo],
                    lhsT=kT[po : po + 64, mt, 128 * i : 128 * (i + 1)],
                    rhs=qT[po : po + 64, mt, lo:hi],
                    start=True,
                    stop=not diag_bank,
                )
                if diag_bank:
                    # block-causal mask: scores -= MASK_C * disallowed
                    mum = mu_sb if half == 0 else mm64[64:72, 0, :]
                    mvm = mv_sb if half == 0 else mm64[64:72, 1, :]
                    nc.tensor.matmul(
                        sp[:, q0 - glo : q0 - glo + 128],
                        lhsT=mum,
                        rhs=mvm,
                        start=False,
                        stop=True,
                    )
            # P^T = exp(rrms_k[k]/sqrt(HD) * scores - B0)
            nc.scalar.activation(
                pt[:, lo_g - glo : 1024],
                sp[:, lo_g - glo : 1024],
                A.Exp,
                bias=b0_sb,
                scale=rkz[:, h, i : i + 1],
            )
            return pt

        def attn_pv(h, qh, i, av, pt):
            """attn^T accumulation (+ denominator in row 64)."""
            glo = 1024 * qh
            kmax = 8 * (qh + 1)
            q0 = 128 * i
            for jj in range(2):
                j = 2 * qh + jj
                jlo = max(512 * j, q0)
                jhi = 512 * (j + 1)
                if jlo >= jhi:
                    continue
                nc.tensor.matmul(
                    av[:, jj, jlo - 512 * j : 512],
                    lhsT=vv[:, i, h, :],
                    rhs=pt[:, jlo - glo : jhi - glo],
                    start=(i == 0),
                    stop=(i == min(kmax, 4 * j + 4) - 1),
                )

        def normalize(h, qh, av, jj):
            """Evacuate one jj half of av to SBUF (its accumulation closed at
            i=4j+3), then divide the 64 head rows by the denominator row.
            Splitting by jj lets half the work run inside the i loop and
            frees the psum bank sooner at group end."""
            mt, half = divmod(h, 2)
            po = 64 * half
            lo = 1024 * qh + 512 * jj
            avs = attnw.tile([65, 512], F32, tag="avs")
            nc.vector.tensor_copy(avs, av[:, jj, :])
            rden = attnw.tile([1, 512], F32, tag="rden")
            nc.vector.reciprocal(rden, avs[64:65])
            rdb = attnw.tile([64, 512], F32, tag="rdb")
            nc.gpsimd.partition_broadcast(rdb, rden, channels=64)
            nc.vector.tensor_mul(
                at[po : po + 64, mt, lo : lo + 512],
                avs[0:64],
                rdb,
            )

        def outproj_m(jh, m):
            """One 128-row tile of the partial out-projection y^T."""
            ps = spp.tile([128, 1024], F32, tag="spp")
            for jj in range(2):
                j = 2 * jh + jj
                for kt in range(2):
                    nc.tensor.matmul(
                        ps[:, 512 * jj : 512 * (jj + 1)],
                        lhsT=wo_sb[:, kt, 128 * m : 128 * (m + 1)],
                        rhs=at[:, kt, 512 * j : 512 * (j + 1)],
                        start=(kt == 0),
                        stop=(kt == 1),
                    )
            yst = ystp.tile([128, 2, 512], F16, tag="yst")
            # jh=0 runs inside the attention window where ACT is the
            # bottleneck -> DVE; jh=1 is the tail where ACT idles -> mostly ACT
            # (DVE carries the trailing normalizes there).
            if jh == 1:
                nc.scalar.copy(yst, ps.rearrange("p (a b) -> p a b", a=2))
            else:
                nc.vector.tensor_copy(yst, ps.rearrange("p (a b) -> p a b", a=2))
            eng = nc.sync if m % 2 == 0 else nc.scalar
            eng.dma_start(
                out=yt_d[4 * m + 2 * jh : 4 * m + 2 * jh + 2].rearrange(
                    "a p b -> p a b"
                ),
                in_=yst,
            )

        # ---- phase 3: attention, qh-outer, head pairs interleaved, with a
        # one-step software pipeline: scores+exp of step N issue before the
        # PV of step N-1, so group boundaries never starve the ACT engine ----
        steps = []
        for qh in range(2):
            for mt in range(2):
                gav = {}
                for i in range(8 * (qh + 1)):
                    for h in (2 * mt, 2 * mt + 1):
                        steps.append((h, qh, mt, i, gav))

        def post_pv(h, qh, mt, i, gav):
            """Hooks that must follow the PV of (h, qh, i)."""
            hA, hB = 2 * mt, 2 * mt + 1
            if h != hB:
                return
            kmax = 8 * (qh + 1)
            jj0_done = 3 if qh == 0 else 11
            if i == jj0_done:
                # first query-half of av closed -> normalize it now
                normalize(hA, qh, gav[hA], 0)
                normalize(hB, qh, gav[hB], 0)
            if qh == 1 and i in (3, 7, 9):
                # interleave the jh=0 out-projection into the qh=1 stream;
                # keep grabs inside the wide-exp region — a grab at i=11
                # starves the small trailing exps of the group
                outproj_m(0, 3 * mt + (3, 7, 9).index(i))
            if i == kmax - 1:
                normalize(hA, qh, gav[hA], 1)
                normalize(hB, qh, gav[hB], 1)

        pending = None
        for h, qh, mt, i, gav in steps:
            pt = attn_se(h, qh, i)
            if pending is not None:
                ph, pqh, pmt, pi, pgav, ppt = pending
                attn_pv(ph, pqh, pi, pgav[ph], ppt)
                post_pv(ph, pqh, pmt, pi, pgav)
            if h not in gav:
                av_t = avp.tile([65, 2, 512], F32, tag="av")
                gav[h] = av_t
            pending = (h, qh, mt, i, gav, pt)
        ph, pqh, pmt, pi, pgav, ppt = pending
        attn_pv(ph, pqh, pi, pgav[ph], ppt)
        post_pv(ph, pqh, pmt, pi, pgav)

        # ---- phase 4: remaining out-projection (jh=1) ----
        for m in range(8):
            outproj_m(1, m)


class _pin_act_table:
    """Force every activation we use (Exp, Ln, Copy, Square) onto the one
    table set containing them all, so the program does a single
    ACT_TABLE_LOAD. Restores the shared cached dict on exit."""

    def __init__(self, arch):
        from concourse.hw_specs import get_activation_tables

        self.tabs = get_activation_tables(arch)

    def __enter__(self):
        self.saved = {nm: set(s) for nm, s in self.tabs.items()}
        for nm, s in self.tabs.items():
            if nm != "natural_log_exp_and_others":
                s.clear()

    def __exit__(self, *a):
        for nm, s in self.tabs.items():
            s.clear()
            s.update(self.saved[nm])


def build_program(iters=1):
    nc = bacc.Bacc(
        "TRN2",
        target_bir_lowering=False,
        debug=False,
        enable_asserts=False,
        num_devices=NCORES,
    )
    with tile.TileContext(nc) as tc:
        io = _declare_io(nc)
        for it in range(iters):
            _emit(tc, io, u=f"_i{it}" if iters > 1 else "")
    with _pin_act_table(nc.m.arch):
        nc.compile()
    return nc


def make_core_inputs(x, qkv_w, out_w, qn_w, kn_w, rope_cos, rope_sin, attention_mask):
    """Host-side shard/layout prep. Returns list of 8 per-core input dicts."""
    x = np.asarray(x, np.float32)
    qkv_w = np.asarray(qkv_w, np.float32)
    out_w = np.asarray(out_w, np.float32)
    qn_w = np.asarray(qn_w, np.float32)
    kn_w = np.asarray(kn_w, np.float32)
    rope_cos = np.asarray(rope_cos, np.float32)
    rope_sin = np.asarray(rope_sin, np.float32)
    am = np.asarray(attention_mask)

    r = qkv_w.reshape(3, H, HD, D)
    csT = rope_cos.T.astype(np.float32)                # (64, S)
    snT = rope_sin.T.astype(np.float32)
    s2 = np.concatenate([-snT[0:32], snT[32:64]], axis=0)  # sign-folded sin
    perm = np.concatenate([np.arange(32, 64), np.arange(0, 32)])

    def fold(tab, w, permute):
        ww = w[perm] if permute else w
        return (tab * ww[:, None]).astype(BF16)        # (64, S)

    csq = fold(csT, qn_w, False)
    snq = fold(s2, qn_w, True)
    csk = fold(csT, kn_w, False)
    snk = fold(s2, kn_w, True)

    # rank-8 factorization of the (128,128) diagonal-block mask
    dis = ~(am[0:128, 0:128].T)                        # dis[k', q'] disallowed
    mu = np.zeros((8, 128), np.float32)
    mv = np.zeros((8, 128), np.float32)
    for t in range(8):
        mu[t] = np.arange(128) // 16 == t
        mv[t] = -MASK_C * dis[16 * t, :]
    ones2 = np.zeros((128, 2), np.float32)
    ones2[0:64, 0] = 1.0
    ones2[64:128, 1] = 1.0
    b0 = float(HD * SCALE * max(1e-30, np.abs(qn_w).max() * np.abs(kn_w).max()))
    b0_t = np.full((128, 1), -b0, np.float32)

    shared = dict(
        csq=csq,
        snq=snq,
        csk=csk,
        snk=snk,
        mu=mu.astype(BF16),
        mv=mv.astype(BF16),
        ones2=ones2.astype(BF16),
        b0=b0_t,
    )
    in_maps = []
    for c in range(NCORES):
        b, g = divmod(c, 4)
        hs = slice(HLOC * g, HLOC * (g + 1))
        m = dict(shared)
        m["xt"] = np.ascontiguousarray(x[b].T).astype(BF16)

        def _wlayout(w):
            # (D, M) -> (128, NDK*M): partition p holds [t, m] = w[t*128+p, m]
            mm = w.shape[1]
            return np.ascontiguousarray(
                w.reshape(-1, 128, mm).transpose(1, 0, 2).reshape(128, -1)
            ).astype(BF16)

        m["wq"] = _wlayout(r[0, hs].transpose(2, 0, 1).reshape(D, 256))
        m["wk"] = _wlayout(r[1, hs].transpose(2, 0, 1).reshape(D, 256))
        m["wv"] = _wlayout(r[2, hs].transpose(2, 0, 1).reshape(D, 256))
        m["wo"] = _wlayout(
            np.ascontiguousarray(out_w[:, 256 * g : 256 * (g + 1)].T)
        )
        in_maps.append(m)
    return in_maps


_PROGRAM = []


def get_program():
    if not _PROGRAM:
        _PROGRAM.append(build_program())
    return _PROGRAM[0]


def unshard(results):
    """results: list of 8 dicts with 'yt' (32, 128, 512) f16 partials."""
    ys = []
    for b in range(B):
        acc = np.zeros((32, 128, 512), np.float32)
        for g in range(4):
            acc += np.asarray(results[4 * b + g]["yt"], np.float32)
        yt = acc.reshape(8, 4, 128, 512).transpose(0, 2, 1, 3).reshape(D, S)
        ys.append(yt.T.astype(np.float32))
    return np.stack(ys)


def kernel(**inputs):
    in_maps = make_core_inputs(**inputs)
    nc = get_program()
    res = run_bass_kernel_spmd(nc, in_maps, core_ids=list(range(NCORES)))
    return unshard(res.results)


# revision 33
# speedup vs baseline: 1.3857x; 1.0023x over previous
"""Block-causal attention (B=2, S=2048, D=1024, H=16, HD=64, BLOCK=16) on 8 TRN2 cores.

Sharding: core c -> batch c//4, head-group c%4 (4 heads). Each core computes the
full attention for its 4 heads plus a partial out-projection y^T (1024, 2048) in
f16; the host sums the 4 partials per batch and transposes.

v2 restructure vs baseline:
  - xt DMA in column blocks (arrival order == consumption order) so the Q0
    projection starts ~2us in.
  - proj order Q0,K0 -> lnexp group0 -> V -> Q1,K1 -> lnexp group1. RMS-norm
    Ln/Exp merged per group: pairs live at 32-partition offsets in one psum
    tile, one Ln + one Exp on [34, 2048] instead of 4 instructions apiece.
  - squares on ACT (Square), psum->sbuf raw copies split DVE/ACT to balance
    engines under the PE roofline.
  - attention qh-outer with the two heads of an mt pair interleaved per
    k-tile: their K=64 score matmuls sit at tile_position (0,0)/(64,0) and
    run concurrently on HW; second mask-factor copy lives at partitions 64-71.
  - av evacuated to SBUF by DVE right after the last PV so the psum bank
    frees early; normalize runs from the SBUF copy.
  - out-proj jh=0 interleaved into the qh=1 attention stream (shared score
    psum pool); output yt in f16 (host sums partials in f32).
"""

import numpy as np
import ml_dtypes

import concourse.bass as bass
import concourse.tile as tile
from concourse import bacc
from concourse import mybir
from concourse.bass_utils import run_bass_kernel_spmd

BF16 = ml_dtypes.bfloat16
F32 = mybir.dt.float32
F16 = mybir.dt.float16
BF = mybir.dt.bfloat16

B, S, D, H, HD = 2, 2048, 1024, 16, 64
HLOC = 4          # heads per core
NCORES = 8
EPS = 1e-6
SCALE = HD ** -0.5
MASK_C = 8192.0   # masked-pair score offset; exp underflows to 0.0
NST = 4           # 512-wide seq tiles
NKT = 16          # 128-wide key tiles
NDK = 8           # 128-wide model-dim tiles


def _declare_io(nc):
    def din(name, shape, d=BF):
        return nc.dram_tensor(name, shape, d, kind="ExternalInput").ap()

    io = dict(
        xt_d=din("xt", [D, S]),
        wq_d=din("wq", [128, NDK * 256]),
        wk_d=din("wk", [128, NDK * 256]),
        wv_d=din("wv", [128, NDK * 256]),
        wo_d=din("wo", [128, 2 * D]),
        csq_d=din("csq", [64, S]),
        snq_d=din("snq", [64, S]),
        csk_d=din("csk", [64, S]),
        snk_d=din("snk", [64, S]),
        mu_d=din("mu", [8, 128]),
        mv_d=din("mv", [8, 128]),
        ones2_d=din("ones2", [128, 2]),
        b0_d=din("b0", [128, 1], F32),
        yt_d=nc.dram_tensor(
            "yt", [32, 128, 512], F16, kind="ExternalOutput"
        ).ap(),
    )
    return io


def _emit(tc, io, u=""):
    """Emit the per-core program. Pure SPMD: identical on all 8 cores."""
    from contextlib import ExitStack

    nc = tc.nc
    A = mybir.ActivationFunctionType
    xt_d = io["xt_d"]
    wo_d = io["wo_d"]
    mu_d = io["mu_d"]
    mv_d = io["mv_d"]
    ones2_d = io["ones2_d"]
    b0_d = io["b0_d"]
    yt_d = io["yt_d"]

    ctx = ExitStack()
    proj_ctx = ExitStack()
    with ctx:
        consts = ctx.enter_context(tc.tile_pool(name="consts" + u, bufs=1))
        persist = ctx.enter_context(tc.tile_pool(name="persist" + u, bufs=1))
        dscratch = ctx.enter_context(
            tc.tile_pool(name="dscratch" + u, bufs=1, space="DRAM")
        )
        xtp = proj_ctx.enter_context(tc.tile_pool(name="xtp" + u, bufs=1))
        work2 = proj_ctx.enter_context(tc.tile_pool(name="work2" + u, bufs=2))
        sqp = proj_ctx.enter_context(tc.tile_pool(name="sqp" + u, bufs=3))
        pp = proj_ctx.enter_context(
            tc.tile_pool(name="pp" + u, bufs=2, space="PSUM")
        )
        msp = proj_ctx.enter_context(
            tc.tile_pool(name="msp" + u, bufs=2, space="PSUM")
        )

        # ---- input DMA: xt in column blocks (2 chunks per 512-col block)
        # on the sync/scalar HWDGE queues; weights+tables on the gpsimd
        # SWDGE queue in consumption order ----
        xt_sb = xtp.tile([128, NDK, S], BF)
        xt_rd = xt_d.rearrange("(a p) c -> p a c", p=128)
        for st in range(NST):
            sl = slice(512 * st, 512 * (st + 1))
            nc.sync.dma_start(out=xt_sb[:, 0:4, sl], in_=xt_rd[:, 0:4, sl])
            nc.scalar.dma_start(out=xt_sb[:, 4:8, sl], in_=xt_rd[:, 4:8, sl])

        wq_sb = consts.tile([128, NDK, 256], BF)
        wk_sb = consts.tile([128, NDK, 256], BF)
        wv_sb = consts.tile([128, NDK, 256], BF)
        wo_sb = consts.tile([128, 2, D], BF)
        for nm, t in (("wv", wv_sb), ("wq", wq_sb), ("wk", wk_sb)):
            nc.gpsimd.dma_start(
                out=t, in_=io[nm + "_d"].rearrange("p (t m) -> p t m", t=NDK)
            )
        # rope tables: DRAM holds 64 rows; duplicate into both SBUF halves
        csq_sb = consts.tile([128, S], BF)
        snq_sb = consts.tile([128, S], BF)
        csk_sb = consts.tile([128, S], BF)
        snk_sb = consts.tile([128, S], BF)
        for nm, t in (
            ("csq", csq_sb), ("snq", snq_sb), ("csk", csk_sb), ("snk", snk_sb)
        ):
            nc.gpsimd.dma_start(out=t[0:64], in_=io[nm + "_d"])
            nc.gpsimd.dma_start(out=t[64:128], in_=io[nm + "_d"])
        # rank-8 mask factors at partitions 0-7 (heads at po=0) and a second
        # copy at partitions 64-71 (heads at po=64) for PE row-group overlap
        mu_sb = consts.tile([8, 128], BF)
        mv_sb = consts.tile([8, 128], BF)
        mm64 = consts.tile([72, 2, 128], BF)
        nc.sync.dma_start(out=mu_sb, in_=mu_d)
        nc.sync.dma_start(out=mv_sb, in_=mv_d)
        nc.sync.dma_start(out=mm64[64:72, 0, :], in_=mu_d)
        nc.sync.dma_start(out=mm64[64:72, 1, :], in_=mv_d)
        ones2_sb = consts.tile([128, 2], BF)
        nc.sync.dma_start(out=ones2_sb, in_=ones2_d)
        b0_sb = consts.tile([128, 1], F32)
        nc.sync.dma_start(out=b0_sb, in_=b0_d)
        eps_sb = consts.tile([128, 1], F32)
        nc.vector.memset(eps_sb, EPS)

        # ---- persistent activations ----
        qT = persist.tile([128, 2, S], BF)      # (2 heads)*64 rows per m-tile
        kT = persist.tile([128, 2, S], BF)
        vv = persist.tile([128, NKT, HLOC, HD + 1], BF)   # [V | ones]
        at = persist.tile([128, 2, S], BF)      # normalized attn^T
        # pair p's two rrms rows live at partition 32*p
        ln8 = persist.tile([98, NST, 512], F32)
        rr8 = persist.tile([98, NST, 512], BF)
        rkb = persist.tile([128, 64], BF)    # k-side rrms, (k mod 128, h*16+i)
        rkz = persist.tile([128, 4, 16], F32)  # SCALE * rrms_k per (head, ktile)
        rr_dram = dscratch.tile([8, 16, 128], BF)

        nc.vector.memset(vv[:, :, :, HD : HD + 1], 1.0)

        # ---- phase 1: Q/K projections + RMS-norm stats + RoPE ----
        def proj_pair(qk_i, mt):
            """Project pair (qk_i: 0=Q, 1=K) for m-tile mt; fill qraw, compute
            rrms = exp(-0.5*ln(ms/HD+eps)) into rr8. Returns the qraw tile."""
            pair = 2 * mt + qk_i
            pb = 32 * pair
            wsb = wq_sb if qk_i == 0 else wk_sb
            qraw = work2.tile([128, S], BF, tag="qraw")
            for hf in range(2):
                ms_t = msp.tile([2, 2, 512], F32, tag="ms")
                for s2 in range(2):
                    st = 2 * hf + s2
                    ps = pp.tile([128, 512], F32, tag="pp")
                    for kt in range(NDK):
                        nc.tensor.matmul(
                            ps,
                            lhsT=wsb[:, kt, 128 * mt : 128 * (mt + 1)],
                            rhs=xt_sb[:, kt, 512 * st : 512 * (st + 1)],
                            start=(kt == 0),
                            stop=(kt == NDK - 1),
                        )
                    sl = slice(512 * st, 512 * (st + 1))
                    if st % 2 == 0:
                        nc.vector.tensor_copy(qraw[:, sl], ps)
                    else:
                        nc.scalar.copy(qraw[:, sl], ps)
                    sq = sqp.tile([128, 512], BF, tag="sq")
                    nc.scalar.activation(sq, qraw[:, sl], A.Square)
                    nc.tensor.matmul(
                        ms_t[:, s2, :],
                        lhsT=ones2_sb,
                        rhs=sq,
                        start=True,
                        stop=True,
                    )
                nc.scalar.activation(
                    ln8[pb : pb + 2, 2 * hf : 2 * hf + 2, :],
                    ms_t,
                    A.Ln,
                    bias=eps_sb[0:2],
                    scale=1.0 / HD,
                )
                nc.scalar.activation(
                    rr8[pb : pb + 2, 2 * hf : 2 * hf + 2, :],
                    ln8[pb : pb + 2, 2 * hf : 2 * hf + 2, :],
                    A.Exp,
                    scale=-0.5,
                )
            return qraw

        def rope(qk_i, mt, qraw):
            """RoPE on raw projection output. Q side: multiply by the
            broadcast rrms afterwards; K side: plain add (rrms folded into
            the exp scale)."""
            pair = 2 * mt + qk_i
            pb = 32 * pair
            cstab = csq_sb if qk_i == 0 else csk_sb
            sntab = snq_sb if qk_i == 0 else snk_sb
            dest = qT if qk_i == 0 else kT
            rot = work2.tile([128, S], BF, tag="rot")
            for lo, hi in ((0, 32), (32, 64), (64, 96), (96, 128)):
                src_lo = lo + 32 if (lo // 32) % 2 == 0 else lo - 32
                eng = nc.sync if lo < 64 else nc.scalar
                eng.dma_start(out=rot[lo:hi], in_=qraw[src_lo : src_lo + 32])
            t1 = work2.tile([128, S], BF, tag="t1")
            t2 = work2.tile([128, S], BF, tag="t2")
            nc.vector.tensor_mul(t1, qraw, cstab)
            nc.vector.tensor_mul(t2, rot, sntab)
            nc.sync.dma_start(
                out=rr_dram[2 * pair : 2 * pair + 2].rearrange("r a b -> r (a b)"),
                in_=rr8[pb : pb + 2].rearrange("p a b -> p (a b)"),
            )
            if qk_i == 1:
                nc.vector.tensor_add(dest[:, mt, :], t1, t2)
                # k-side rrms rows -> partition-major via DMA transpose,
                # folding in the 1/sqrt(HD) softmax scale
                nc.sync.dma_start_transpose(
                    rkb[:, 32 * mt : 32 * (mt + 1)],
                    rr_dram[2 * pair : 2 * pair + 2].rearrange("r a b -> (r a) b"),
                )
                nc.vector.tensor_scalar_mul(
                    rkz[:, 2 * mt : 2 * mt + 2, :].rearrange("p h i -> p (h i)"),
                    rkb[:, 32 * mt : 32 * (mt + 1)],
                    SCALE,
                )
            else:
                tsum = work2.tile([128, S], BF, tag="tsum")
                nc.vector.tensor_add(tsum, t1, t2)
                rrb = work2.tile([128, NST, 512], BF, tag="rrb")
                nc.gpsimd.dma_start(
                    out=rrb[0:64],
                    in_=rr_dram[2 * pair : 2 * pair + 1]
                    .rearrange("r a b -> r (a b)")
                    .rearrange("r (a b) -> r a b", a=NST)
                    .partition_broadcast(64),
                )
                nc.gpsimd.dma_start(
                    out=rrb[64:128],
                    in_=rr_dram[2 * pair + 1 : 2 * pair + 2]
                    .rearrange("r a b -> r (a b)")
                    .rearrange("r (a b) -> r a b", a=NST)
                    .partition_broadcast(64),
                )
                for st in range(NST):
                    sl = slice(512 * st, 512 * (st + 1))
                    nc.vector.tensor_mul(
                        dest[:, mt, sl], tsum[:, sl], rrb[:, st, :]
                    )

        # ---- phase 1a: V projection first — it only needs xt, so it rides
        # the tail of the input DMA; two st tiles per psum bank ----
        for sp2 in range(NKT // 2):
            ps = pp.tile([128, 512], F32, tag="pp")
            for half in range(2):
                stv = 2 * sp2 + half
                for kt in range(NDK):
                    nc.tensor.matmul(
                        ps[:, 256 * half : 256 * (half + 1)],
                        lhsT=xt_sb[:, kt, 128 * stv : 128 * (stv + 1)],
                        rhs=wv_sb[:, kt, :],
                        start=(kt == 0),
                        stop=(kt == NDK - 1),
                    )
            nc.vector.tensor_copy(
                vv[:, 2 * sp2 : 2 * sp2 + 2, :, 0:HD],
                ps.rearrange("p (a h d) -> p a h d", a=2, h=HLOC),
            )

        for mt in range(2):
            qraw_q = proj_pair(0, mt)
            rope(0, mt, qraw_q)
            qraw_k = proj_pair(1, mt)
            rope(1, mt, qraw_k)

        # wo is first needed by the out-proj grabs deep in the attention
        # phase; loading it here keeps the startup DMA burst shorter
        nc.gpsimd.dma_start(out=wo_sb, in_=wo_d.rearrange("p (t m) -> p t m", t=2))

        # proj scratch (incl. x^T) is dead now; free SBUF/PSUM for attention
        proj_ctx.close()
        attnw = ctx.enter_context(tc.tile_pool(name="attnw" + u, bufs=3))
        ptp = ctx.enter_context(tc.tile_pool(name="ptp" + u, bufs=6))
        ystp = ctx.enter_context(tc.tile_pool(name="ystp" + u, bufs=4))
        spp = ctx.enter_context(tc.tile_pool(name="spp" + u, bufs=2, space="PSUM"))
        avp = ctx.enter_context(tc.tile_pool(name="avp" + u, bufs=2, space="PSUM"))

        def attn_se(h, qh, i):
            """Scores + exp for head h, query-half qh, k-tile i -> pt."""
            mt, half = divmod(h, 2)
            po = 64 * half
            glo = 1024 * qh
            q0 = 128 * i
            lo_g = max(glo, q0)
            pt = ptp.tile([128, 1024], BF, tag="pt")
            sp = spp.tile([128, 1024], F32, tag="spp")
            has_diag = glo <= q0 < glo + 1024
            for jj in range(2):
                j = 2 * qh + jj
                lo = max(512 * j, q0)
                hi = 512 * (j + 1)
                if lo >= hi:
                    continue
                diag_bank = has_diag and (q0 - glo) // 512 == jj
                nc.tensor.matmul(
                    sp[:, lo - glo : hi - glo],
                    lhsT=kT[po : po + 64, mt, 128 * i : 128 * (i + 1)],
                    rhs=qT[po : po + 64, mt, lo:hi],
                    start=True,
                    stop=not diag_bank,
                )
                if diag_bank:
                    # block-causal mask: scores -= MASK_C * disallowed
                    mum = mu_sb if half == 0 else mm64[64:72, 0, :]
                    mvm = mv_sb if half == 0 else mm64[64:72, 1, :]
                    nc.tensor.matmul(
                        sp[:, q0 - glo : q0 - glo + 128],
                        lhsT=mum,
                        rhs=mvm,
                        start=False,
                        stop=True,
                    )
            # P^T = exp(rrms_k[k]/sqrt(HD) * scores - B0)
            nc.scalar.activation(
                pt[:, lo_g - glo : 1024],
                sp[:, lo_g - glo : 1024],
                A.Exp,
                bias=b0_sb,
                scale=rkz[:, h, i : i + 1],
            )
            return pt

        def attn_pv(h, qh, i, av, pt):
            """attn^T accumulation (+ denominator in row 64)."""
            glo = 1024 * qh
            kmax = 8 * (qh + 1)
            q0 = 128 * i
            for jj in range(2):
                j = 2 * qh + jj
                jlo = max(512 * j, q0)
                jhi = 512 * (j + 1)
                if jlo >= jhi:
                    continue
                nc.tensor.matmul(
                    av[:, jj, jlo - 512 * j : 512],
                    lhsT=vv[:, i, h, :],
                    rhs=pt[:, jlo - glo : jhi - glo],
                    start=(i == 0),
                    stop=(i == min(kmax, 4 * j + 4) - 1),
                )

        def normalize(h, qh, av, jj):
            """Evacuate one jj half of av to SBUF (its accumulation closed at
            i=4j+3), then divide the 64 head rows by the denominator row.
            Splitting by jj lets half the work run inside the i loop and
            frees the psum bank sooner at group end."""
            mt, half = divmod(h, 2)
            po = 64 * half
            lo = 1024 * qh + 512 * jj
            avs = attnw.tile([65, 512], F32, tag="avs")
            nc.vector.tensor_copy(avs, av[:, jj, :])
            rden = attnw.tile([1, 512], F32, tag="rden")
            nc.vector.reciprocal(rden, avs[64:65])
            rdb = attnw.tile([64, 512], F32, tag="rdb")
            nc.gpsimd.partition_broadcast(rdb, rden, channels=64)
            nc.vector.tensor_mul(
                at[po : po + 64, mt, lo : lo + 512],
                avs[0:64],
                rdb,
            )

        def outproj_m(jh, m):
            """One 128-row tile of the partial out-projection y^T."""
            ps = spp.tile([128, 1024], F32, tag="spp")
            for jj in range(2):
                j = 2 * jh + jj
                for kt in range(2):
                    nc.tensor.matmul(
                        ps[:, 512 * jj : 512 * (jj + 1)],
                        lhsT=wo_sb[:, kt, 128 * m : 128 * (m + 1)],
                        rhs=at[:, kt, 512 * j : 512 * (j + 1)],
                        start=(kt == 0),
                        stop=(kt == 1),
                    )
            yst = ystp.tile([128, 2, 512], F16, tag="yst")
            # jh=0 runs inside the attention window where ACT is the
            # bottleneck -> DVE; jh=1 is the tail where ACT idles -> mostly ACT
            # (DVE carries the trailing normalizes there).
            if jh == 1:
                nc.scalar.copy(yst, ps.rearrange("p (a b) -> p a b", a=2))
            else:
                nc.vector.tensor_copy(yst, ps.rearrange("p (a b) -> p a b", a=2))
            eng = nc.sync if m % 2 == 0 else nc.scalar
            eng.dma_start(
                out=yt_d[4 * m + 2 * jh : 4 * m + 2 * jh + 2].rearrange(
                    "a p b -> p a b"
                ),
                in_=yst,
            )

        # ---- phase 3: attention, qh-outer, head pairs interleaved, with a
        # one-step software pipeline: scores+exp of step N issue before the
        # PV of step N-1, so group boundaries never starve the ACT engine ----
        steps = []
        for qh in range(2):
            for mt in range(2):
                gav = {}
                for i in range(8 * (qh + 1)):
                    for h in (2 * mt, 2 * mt + 1):
                        steps.append((h, qh, mt, i, gav))

        def post_pv(h, qh, mt, i, gav):
            """Hooks that must follow the PV of (h, qh, i)."""
            hA, hB = 2 * mt, 2 * mt + 1
            if h != hB:
                return
            kmax = 8 * (qh + 1)
            jj0_done = 3 if qh == 0 else 11
            if i == jj0_done:
                # first query-half of av closed -> normalize it now
                normalize(hA, qh, gav[hA], 0)
                normalize(hB, qh, gav[hB], 0)
            if qh == 1 and i in (3, 7, 9):
                # interleave the jh=0 out-projection into the qh=1 stream;
                # keep grabs inside the wide-exp region — a grab at i=11
                # starves the small trailing exps of the group
                outproj_m(0, 3 * mt + (3, 7, 9).index(i))
            if i == kmax - 1:
                normalize(hA, qh, gav[hA], 1)
                normalize(hB, qh, gav[hB], 1)

        from collections import deque

        pend = deque()

        def flush_one():
            ph, pqh, pmt, pi, pgav, ppt = pend.popleft()
            attn_pv(ph, pqh, pi, pgav[ph], ppt)
            post_pv(ph, pqh, pmt, pi, pgav)

        for h, qh, mt, i, gav in steps:
            pt = attn_se(h, qh, i)
            if len(pend) >= 2:
                flush_one()
            if h not in gav:
                av_t = avp.tile([65, 2, 512], F32, tag="av")
                gav[h] = av_t
            pend.append((h, qh, mt, i, gav, pt))
        while pend:
            flush_one()

        # ---- phase 4: remaining out-projection (jh=1) ----
        for m in range(8):
            outproj_m(1, m)


class _pin_act_table:
    """Force every activation we use (Exp, Ln, Copy, Square) onto the one
    table set containing them all, so the program does a single
    ACT_TABLE_LOAD. Restores the shared cached dict on exit."""

    def __init__(self, arch):
        from concourse.hw_specs import get_activation_tables

        self.tabs = get_activation_tables(arch)

    def __enter__(self):
        self.saved = {nm: set(s) for nm, s in self.tabs.items()}
        for nm, s in self.tabs.items():
            if nm != "natural_log_exp_and_others":
                s.clear()

    def __exit__(self, *a):
        for nm, s in self.tabs.items():
            s.clear()
            s.update(self.saved[nm])


def build_program(iters=1):
    nc = bacc.Bacc(
        "TRN2",
        target_bir_lowering=False,
        debug=False,
        enable_asserts=False,
        num_devices=NCORES,
    )
    with tile.TileContext(nc) as tc:
        io = _declare_io(nc)
        for it in range(iters):
            _emit(tc, io, u=f"_i{it}" if iters > 1 else "")
    with _pin_act_table(nc.m.arch):
        nc.compile()
    return nc


def make_core_inputs(x, qkv_w, out_w, qn_w, kn_w, rope_cos, rope_sin, attention_mask):
    """Host-side shard/layout prep. Returns list of 8 per-core input dicts."""
    x = np.asarray(x, np.float32)
    qkv_w = np.asarray(qkv_w, np.float32)
    out_w = np.asarray(out_w, np.float32)
    qn_w = np.asarray(qn_w, np.float32)
    kn_w = np.asarray(kn_w, np.float32)
    rope_cos = np.asarray(rope_cos, np.float32)
    rope_sin = np.asarray(rope_sin, np.float32)
    am = np.asarray(attention_mask)

    r = qkv_w.reshape(3, H, HD, D)
    csT = rope_cos.T.astype(np.float32)                # (64, S)
    snT = rope_sin.T.astype(np.float32)
    s2 = np.concatenate([-snT[0:32], snT[32:64]], axis=0)  # sign-folded sin
    perm = np.concatenate([np.arange(32, 64), np.arange(0, 32)])

    def fold(tab, w, permute):
        ww = w[perm] if permute else w
        return (tab * ww[:, None]).astype(BF16)        # (64, S)

    csq = fold(csT, qn_w, False)
    snq = fold(s2, qn_w, True)
    csk = fold(csT, kn_w, False)
    snk = fold(s2, kn_w, True)

    # rank-8 factorization of the (128,128) diagonal-block mask
    dis = ~(am[0:128, 0:128].T)                        # dis[k', q'] disallowed
    mu = np.zeros((8, 128), np.float32)
    mv = np.zeros((8, 128), np.float32)
    for t in range(8):
        mu[t] = np.arange(128) // 16 == t
        mv[t] = -MASK_C * dis[16 * t, :]
    ones2 = np.zeros((128, 2), np.float32)
    ones2[0:64, 0] = 1.0
    ones2[64:128, 1] = 1.0
    b0 = float(HD * SCALE * max(1e-30, np.abs(qn_w).max() * np.abs(kn_w).max()))
    b0_t = np.full((128, 1), -b0, np.float32)

    shared = dict(
        csq=csq,
        snq=snq,
        csk=csk,
        snk=snk,
        mu=mu.astype(BF16),
        mv=mv.astype(BF16),
        ones2=ones2.astype(BF16),
        b0=b0_t,
    )
    in_maps = []
    for c in range(NCORES):
        b, g = divmod(c, 4)
        hs = slice(HLOC * g, HLOC * (g + 1))
        m = dict(shared)
        m["xt"] = np.ascontiguousarray(x[b].T).astype(BF16)

        def _wlayout(w):
            # (D, M) -> (128, NDK*M): partition p holds [t, m] = w[t*128+p, m]
            mm = w.shape[1]
            return np.ascontiguousarray(
                w.reshape(-1, 128, mm).transpose(1, 0, 2).reshape(128, -1)
            ).astype(BF16)

        m["wq"] = _wlayout(r[0, hs].transpose(2, 0, 1).reshape(D, 256))
        m["wk"] = _wlayout(r[1, hs].transpose(2, 0, 1).reshape(D, 256))
        m["wv"] = _wlayout(r[2, hs].transpose(2, 0, 1).reshape(D, 256))
        m["wo"] = _wlayout(
            np.ascontiguousarray(out_w[:, 256 * g : 256 * (g + 1)].T)
        )
        in_maps.append(m)
    return in_maps


_PROGRAM = []


def get_program():
    if not _PROGRAM:
        _PROGRAM.append(build_program())
    return _PROGRAM[0]


def unshard(results):
    """results: list of 8 dicts with 'yt' (32, 128, 512) f16 partials."""
    ys = []
    for b in range(B):
        acc = np.zeros((32, 128, 512), np.float32)
        for g in range(4):
            acc += np.asarray(results[4 * b + g]["yt"], np.float32)
        yt = acc.reshape(8, 4, 128, 512).transpose(0, 2, 1, 3).reshape(D, S)
        ys.append(yt.T.astype(np.float32))
    return np.stack(ys)


def kernel(**inputs):
    in_maps = make_core_inputs(**inputs)
    nc = get_program()
    res = run_bass_kernel_spmd(nc, in_maps, core_ids=list(range(NCORES)))
    return unshard(res.results)
